# revision 9
# baseline (speedup 1.0000x reference)
"""Self-contained Trainium2 Bass kernel for the DecoConv GNN layer.

kernel(**inputs) takes the full (unsharded) numpy inputs and returns the full
[100000, 64] fp32 output. Internally: shards destination nodes across the 8
NeuronCores, builds + compiles one SPMD Bass/Tile program on first call, and
runs it via concourse's PJRT path on cores 0-7.
"""
import sys
if '/opt/trn_rl_repo' not in sys.path:
    sys.path.insert(0, '/opt/trn_rl_repo')

import numpy as np

# ======================================================================
# environment fixups (walrus single-sync-wait limit, NTFF hook, uploads)
# ======================================================================
"""Split multi-wait instructions in BIR JSON: this container's walrus supports
only ONE sync wait per instruction. Extra waits are moved onto standalone
EventSemaphore instructions inserted immediately before (same engine, in-order)."""
import orjson

# opcodes that must stay glued to the following instruction (weights load + matmul)
_GLUE_PREV = {"TensorLoad", "LoadStationary", "TensorLoadWeights", "LdWeights"}

def split_multiwaits_json(bir_bytes: bytes) -> bytes:
    d = orjson.loads(bir_bytes)
    n_split = 0
    uid = [0]
    for fn in d.get("functions", []):
        for blk in fn.get("blocks", []):
            insts = blk.get("instructions", [])
            out = []
            for inst in insts:
                si = inst.get("sync_info") or {}
                waits = si.get("on_wait") or []
                if len(waits) > 1:
                    n_split += 1
                    pre = []
                    for w in waits:
                        uid[0] += 1
                        pre.append({
                            "debug": inst.get("debug", 0),
                            "engine": inst["engine"],
                            "ins": [],
                            "name": f"{inst['name']}_sw{uid[0]}",
                            "opcode": "EventSemaphore",
                            "outs": [],
                            "sync_info": {"on_update": [], "on_wait": [w]},
                        })
                    si["on_wait"] = []
                    inst["sync_info"] = si
                    # insert before a glued weights-load if present
                    ip = len(out)
                    while ip > 0 and out[ip-1].get("opcode") in _GLUE_PREV and out[ip-1].get("engine") == inst["engine"]:
                        ip -= 1
                    out[ip:ip] = pre
                out.append(inst)
            blk["instructions"] = out
    return orjson.dumps(d), n_split

_installed = False

def _make_ntff_hook(so_path="/opt/axon/libaxon_pjrt.so"):
    import contextlib, ctypes
    lib = ctypes.CDLL(so_path)
    if not hasattr(lib, "axon_start_nrt_profile"):
        return None
    lib.axon_start_nrt_profile.argtypes = [ctypes.POINTER(ctypes.c_int64), ctypes.c_size_t]
    lib.axon_start_nrt_profile.restype = ctypes.c_int64
    lib.axon_stop_nrt_profile.argtypes = [ctypes.c_char_p]
    lib.axon_stop_nrt_profile.restype = ctypes.c_int64

    @contextlib.contextmanager
    def _hook(output_dir, device_ids):
        import jax
        jax.devices()
        if device_ids:
            ids = (ctypes.c_int64 * len(device_ids))(*device_ids)
            rc = lib.axon_start_nrt_profile(ids, len(device_ids))
        else:
            rc = lib.axon_start_nrt_profile(None, 0)
        if rc != 0:
            raise RuntimeError(f"axon_start_nrt_profile rc={rc}")
        try:
            yield
        finally:
            n = lib.axon_stop_nrt_profile(str(output_dir).encode())
            if n < 0:
                raise RuntimeError(f"axon_stop_nrt_profile rc={n}")
    return _hook


def install():
    global _installed
    if _installed:
        return
    from concourse import bass2jax, bass_utils
    orig = bass_utils.compile_bir_kernel
    def patched(ant_bir_str, compile_dir_path, neff_name, **kw):
        fixed, n = split_multiwaits_json(ant_bir_str if isinstance(ant_bir_str, bytes) else ant_bir_str.encode())
        return orig(fixed, compile_dir_path, neff_name=neff_name, **kw)
    bass2jax.compile_bir_kernel = patched

    # antenv.axon_hooks shim so run_bass_kernel_spmd(trace=True) works
    import sys, types
    try:
        import antenv.axon_hooks  # noqa
    except ImportError:
        hook = _make_ntff_hook()
        mod = types.ModuleType("antenv.axon_hooks")
        mod.get_axon_ntff_profile_hook = lambda: hook
        mod.set_axon_ntff_profile_hook = lambda h: None
        sys.modules["antenv.axon_hooks"] = mod
        import antenv
        antenv.axon_hooks = mod

    # no-op the artifact upload (no bucket access in this sandbox)
    bass_utils.upload_artifacts = lambda tmpdir: f"local:{tmpdir}"
    _installed = True


# ======================================================================
# kernel build + host pre/post processing
# ======================================================================
"""GNN message-passing kernel for TRN2 (dest-sharded SpMM + Linear + residual + BN + ReLU).

Layout strategy (v2):
- 784 global dest tiles of 128 rows (incl 2 empty pad tiles), snake-dealt to
  the 8 cores by edge count (98 tiles each) so per-position block counts are
  balanced; one shared SPMD program (per-position counts = max across cores).
- Tiles processed in groups of G=4 (25 groups). Per (group, bucket-of-25000
  source rows) one merged dma_gather (queue = bucket -> own Q7 core pair)
  fetches the section's edge slots (128B bf16 features in a 256B slot).
- DVE builds the one-hot M[e, i] = (r_e == i) per section against a dense
  materialized iota (step-1 operand first for 2x mode) and writes
  y = val * x_e into the padded half of each gathered slot.
- TensorE accumulates h1^T[d, i] += y_k^T @ M_k per dest tile in PSUM
  (features on partitions). Linear per group is a single N=512 matmul with a
  bias ones-row; residual add + BN stats (free-dim reductions + tiny
  AllReduce) and the fused scale/shift/ReLU run on DVE/ACT; output is bf16
  (host converts to fp32).
"""

import numpy as np
import ml_dtypes

BF16 = ml_dtypes.bfloat16
FP8 = ml_dtypes.float8_e4m3
D = 64
TILE = 128
XROW = 128          # padded bf16 row length of x in HBM (256 bytes)
BK = 25000          # source-bucket rows (int16 index range)
NBUCK = 4
GTILES = 4          # tiles per group (gather granularity; linear N=512)
N_GLOBAL_TILES = 784  # 782 real (100000/128 rounded up) + 2 pad


# ---------------------------------------------------------------- host prep

def host_prep(x, adj_val, adj_row, adj_col, W, b, n_cores):
    N = x.shape[0]
    assert N == 100000 and n_cores == 8
    n_tiles = N_GLOBAL_TILES // n_cores          # 98 per core
    S_pad = n_tiles * TILE                        # 12544
    n_groups = (n_tiles + GTILES - 1) // GTILES   # 25 (24x4 + 1x2)

    adj_row = np.asarray(adj_row)
    adj_col = np.asarray(adj_col)
    adj_val = np.asarray(adj_val)

    gt = adj_row // TILE                          # global tile of each edge
    cnt_g = np.bincount(gt, minlength=N_GLOBAL_TILES)

    # snake-deal global tiles (desc by count) to cores
    order_g = np.argsort(-cnt_g, kind="stable")
    core_tiles = np.empty((n_cores, n_tiles), dtype=np.int64)
    for p in range(n_tiles):
        blockk = order_g[p * n_cores:(p + 1) * n_cores]
        if p % 2 == 0:
            core_tiles[:, p] = blockk
        else:
            core_tiles[:, p] = blockk[::-1]
    core_of_tile = np.empty(N_GLOBAL_TILES, dtype=np.int64)
    pos_of_tile = np.empty(N_GLOBAL_TILES, dtype=np.int64)
    for c in range(n_cores):
        core_of_tile[core_tiles[c]] = c
        pos_of_tile[core_tiles[c]] = np.arange(n_tiles)

    ecore = core_of_tile[gt]
    epos = pos_of_tile[gt]
    ebuck = adj_col // BK

    # per (core, pos, bucket) counts -> shared block structure
    cnt3 = np.zeros((n_cores, n_tiles, NBUCK), dtype=np.int64)
    np.add.at(cnt3, (ecore, epos, ebuck), 1)
    nb_shared = (cnt3.max(0) + TILE - 1) // TILE          # [n_tiles, NBUCK]
    empty = nb_shared.sum(1) == 0
    nb_shared[empty, 0] = 1

    # block bases: group-major, bucket-major inside group, pos-major inside bucket
    block_base = np.zeros((n_tiles, NBUCK), dtype=np.int64)
    group_of_pos = np.arange(n_tiles) // GTILES
    B = 0
    grp_meta = []   # per group: dict(sect_nb[q], sect_base[q], tiles(pos list), blk_base)
    for g in range(n_groups):
        poss = [p for p in range(n_tiles) if group_of_pos[p] == g]
        gbase = B
        sect_nb = []
        sect_base = []
        for q in range(NBUCK):
            sect_base.append(B)
            for p in poss:
                block_base[p, q] = B
                B += nb_shared[p, q]
            sect_nb.append(B - sect_base[-1])
        grp_meta.append(dict(poss=poss, sect_nb=sect_nb, sect_base=sect_base,
                             blk_base=gbase, nblocks=B - gbase))
    max_grp_blocks = max(m["nblocks"] for m in grp_meta)
    max_sect_nb = max(max(m["sect_nb"]) for m in grp_meta)

    # slot assignment per core
    per_core = []
    for c in range(n_cores):
        m = ecore == c
        ep = epos[m]; eq = ebuck[m]
        ev = adj_val[m]; er = adj_row[m]; ec = adj_col[m]
        key = ep * NBUCK + eq
        sidx = np.argsort(key, kind="stable")
        ep = ep[sidx]; eq = eq[sidx]; ev = ev[sidx]; er = er[sidx]; ec = ec[sidx]
        kk = key[sidx]
        cnt_k = np.bincount(kk, minlength=n_tiles * NBUCK)
        start = np.zeros(n_tiles * NBUCK, dtype=np.int64)
        start[1:] = np.cumsum(cnt_k)[:-1]
        rank = np.arange(len(kk)) - start[kk]
        blk = block_base[ep, eq] + rank // TILE
        lane = rank % TILE

        val_arr = np.zeros((TILE, B), dtype=BF16)
        m8 = np.zeros((TILE, B * TILE), dtype=FP8)
        idx16 = np.zeros((TILE, B * 8), dtype=np.int16)

        val_arr[lane, blk] = ev.astype(BF16)
        rloc = (er - core_tiles[c][ep] * TILE).astype(np.int64)
        m8[lane, blk * TILE + rloc] = FP8(1.0)
        rel = (ec - eq * BK).astype(np.int16)
        colpos = blk * 8 + lane // 16
        rowpos = lane % 16
        for gg in range(8):
            idx16[rowpos + 16 * gg, colpos] = rel

        # transposed residual input + masked ones row (position order)
        xres = np.zeros((D, S_pad), dtype=BF16)
        ones_row = np.zeros((1, S_pad), dtype=BF16)
        for p in range(n_tiles):
            t = core_tiles[c][p]
            lo = t * TILE
            hi = min(lo + TILE, N)
            nvalid = max(0, hi - lo)
            if nvalid > 0:
                xres[:, p * TILE:p * TILE + nvalid] = \
                    np.asarray(x[lo:hi]).T.astype(BF16)
                ones_row[0, p * TILE:p * TILE + nvalid] = 1.0

        per_core.append(dict(val=val_arr, m8=m8, idx16=idx16, xres=xres,
                             ones=ones_row))

    waug = np.zeros((D + 1, D), dtype=BF16)
    waug[:D] = np.asarray(W, dtype=np.float32).T
    waug[D] = np.asarray(b, dtype=np.float32)
    x_pad = np.zeros((N, XROW), dtype=BF16)
    x_pad[:, :D] = np.asarray(x).astype(BF16)

    meta = dict(N=N, n_tiles=n_tiles, S_pad=S_pad, B=B, n_groups=n_groups,
                nb_shared=nb_shared.tolist(),
                block_base=block_base.tolist(),
                grp_meta=grp_meta,
                max_grp_blocks=max_grp_blocks,
                max_sect_nb=max_sect_nb,
                core_tiles=core_tiles.tolist())
    return meta, per_core, waug, x_pad


def host_post(results, metas, n_cores):
    """Assemble full [N, 64] fp32 output from per-core transposed bf16 outputs."""
    meta = metas["meta"]
    n_tiles = meta["n_tiles"]
    N = meta["N"]
    core_tiles = np.asarray(meta["core_tiles"])
    out = np.empty((N, D), dtype=np.float32)
    for c in range(n_cores):
        dev = np.asarray(results[c])  # [64, S_pad] bf16
        for p in range(n_tiles):
            t = core_tiles[c][p]
            lo = t * TILE
            hi = min(lo + TILE, N)
            if hi > lo:
                out[lo:hi] = dev[:, p * TILE:p * TILE + (hi - lo)].T.astype(np.float32)
    return out


# ---------------------------------------------------------------- device build

def build_nc(meta, n_cores, eps, replica_groups=None):
    from concourse import bass, mybir, tile

    N = meta["N"]
    S_pad = meta["S_pad"]
    n_tiles = meta["n_tiles"]
    B = meta["B"]
    nb_shared = meta["nb_shared"]
    block_base = meta["block_base"]
    grp_meta = meta["grp_meta"]
    n_groups = meta["n_groups"]
    max_grp_blocks = meta["max_grp_blocks"]
    max_sect_nb = meta["max_sect_nb"]
    f32 = mybir.dt.float32
    bf16 = mybir.dt.bfloat16
    i16 = mybir.dt.int16
    i32 = mybir.dt.int32

    nc = bass.Bass(debug=False, num_swdge_queues=4)
    x_d = nc.declare_dram_parameter("x_pad", [N, XROW], bf16, isOutput=False)
    idx_d = nc.declare_dram_parameter("idx16", [TILE, B * 8], i16, isOutput=False)
    val_d = nc.declare_dram_parameter("val", [TILE, B], bf16, isOutput=False)
    m8_d = nc.declare_dram_parameter("m8", [TILE, B * TILE], mybir.dt.float8e4, isOutput=False)
    xres_d = nc.declare_dram_parameter("xres", [D, S_pad], bf16, isOutput=False)
    ones_d = nc.declare_dram_parameter("ones", [1, S_pad], bf16, isOutput=False)
    waug_d = nc.declare_dram_parameter("waug", [D + 1, D], bf16, isOutput=False)
    gam_d = nc.declare_dram_parameter("gam", [D, 1], f32, isOutput=False)
    bet_d = nc.declare_dram_parameter("bet", [D, 1], f32, isOutput=False)
    out_d = nc.declare_dram_parameter("outp", [D, S_pad], bf16, isOutput=True)

    cc_in = nc.dram_tensor("cc_in", [D, 2], f32)
    cc_out = nc.dram_tensor("cc_out", [D, 2], f32, addr_space="Shared")

    with tile.TileContext(nc) as tc:
        with (
            tc.tile_pool(name="const", bufs=1) as constp,
            tc.tile_pool(name="big", bufs=1) as bigp,
            tc.tile_pool(name="xgp", bufs=3) as xgp,
            tc.tile_pool(name="mmp", bufs=3) as mmp,
            tc.tile_pool(name="idxp", bufs=3) as idxp,
            tc.tile_pool(name="sqp", bufs=2) as sqp,
            tc.tile_pool(name="h1p", bufs=2) as h1p,
            tc.tile_pool(name="psA", bufs=4, space="PSUM") as psA,
            tc.tile_pool(name="psB", bufs=2, space="PSUM") as psB,
        ):
            val_sb = bigp.tile([TILE, B], bf16)
            xres_sb = bigp.tile([D, S_pad], bf16)
            h3_sb = bigp.tile([D, S_pad], bf16)
            ones_sb = bigp.tile([1, S_pad], bf16)
            waug_sb = constp.tile([D + 1, D], bf16)
            brow_sb = constp.tile([1, D], bf16)
            gam_sb = constp.tile([D, 1], f32)
            bet_sb = constp.tile([D, 1], f32)
            stat_s = constp.tile([D, n_groups], f32)
            stat_q = constp.tile([D, n_groups], f32)

            nc.scalar.dma_start(val_sb[:], val_d[:])
            nc.scalar.dma_start(xres_sb[:], xres_d[:])
            nc.scalar.dma_start(ones_sb[:], ones_d[:])
            nc.scalar.dma_start(waug_sb[:], waug_d[:])
            nc.scalar.dma_start(brow_sb[:], waug_d[D:D + 1, :])
            nc.scalar.dma_start(gam_sb[:], gam_d[:])
            nc.scalar.dma_start(bet_sb[:], bet_d[:])


            # one register per distinct gather size
            nidx_regs = {}
            for g in range(n_groups):
                for q in range(NBUCK):
                    nbs = grp_meta[g]["sect_nb"][q]
                    if nbs and nbs * TILE not in nidx_regs:
                        nidx_regs[nbs * TILE] = nc.gpsimd.to_reg(nbs * TILE)

            for g in range(n_groups):
                gm = grp_meta[g]
                poss = gm["poss"]
                gbase = gm["blk_base"]
                gnb = gm["nblocks"]
                ncols = len(poss) * TILE

                xg = xgp.tile([TILE, max_grp_blocks * XROW], bf16, tag="xg")
                mm = mmp.tile([TILE, max_grp_blocks * TILE], mybir.dt.float8e4, tag="mm")
                idx_sb = idxp.tile([TILE, max_grp_blocks * 8], i16, tag="idx")
                h1 = h1p.tile([D, GTILES * TILE], bf16, tag="h1")

                nc.sync.dma_start(idx_sb[:, :gnb * 8],
                                  idx_d[:, gbase * 8:(gbase + gnb) * 8])
                nc.sync.dma_start(mm[:, :gnb * TILE],
                                  m8_d[:, gbase * TILE:(gbase + gnb) * TILE])

                for q in range(NBUCK):
                    nbs = gm["sect_nb"][q]
                    if nbs == 0:
                        continue
                    rel = gm["sect_base"][q] - gbase
                    nrow = min(BK, N - q * BK)
                    nc.gpsimd.dma_gather(
                        out_ap=xg[:, rel * XROW:(rel + nbs) * XROW].rearrange(
                            "p (b e) -> p b e", e=XROW),
                        in_ap=x_d[q * BK:q * BK + nrow, :],
                        idxs_ap=idx_sb[:, rel * 8:(rel + nbs) * 8],
                        num_idxs=nbs * TILE,
                        num_idxs_reg=nidx_regs[nbs * TILE],
                        elem_size=XROW,
                        elem_step=XROW,
                        queue_num=q,
                        single_packet=False,
                    )

                for q in range(NBUCK):
                    nbs = gm["sect_nb"][q]
                    if nbs == 0:
                        continue
                    rel = gm["sect_base"][q] - gbase
                    sbase = gm["sect_base"][q]
                    # y = val * x into the padded half of each slot
                    nc.vector.tensor_tensor(
                        out=xg[:, rel * XROW:(rel + nbs) * XROW].rearrange(
                            "p (b e) -> p b e", e=XROW)[:, :, D:2 * D],
                        in0=xg[:, rel * XROW:(rel + nbs) * XROW].rearrange(
                            "p (b e) -> p b e", e=XROW)[:, :, 0:D],
                        in1=val_sb[:, sbase:sbase + nbs].unsqueeze(2).to_broadcast(
                            [TILE, nbs, D]),
                        op=bass.mybir.AluOpType.mult,
                    )

                for j, p in enumerate(poss):
                    ps = psA.tile([D, TILE], f32, tag="ps")
                    blks = []
                    for q in range(NBUCK):
                        nb = nb_shared[p][q]
                        bb = block_base[p][q]
                        for k in range(nb):
                            blks.append(bb - gbase + k)
                    for ki, rb in enumerate(blks):
                        nc.tensor.matmul(
                            ps[:],
                            lhsT=xg[:, rb * XROW + D:rb * XROW + 2 * D],
                            rhs=mm[:, rb * TILE:(rb + 1) * TILE],
                            start=(ki == 0),
                            stop=(ki == len(blks) - 1),
                        )
                    nc.scalar.copy(h1[0:D, j * TILE:(j + 1) * TILE], ps[:])

                goff = poss[0] * TILE
                ps2 = psB.tile([D, GTILES * TILE], f32, tag="ps2")
                nc.tensor.matmul(
                    ps2[:, :ncols],
                    lhsT=waug_sb[0:D, :],
                    rhs=h1[:, :ncols],
                    start=True, stop=False,
                )
                nc.tensor.matmul(
                    ps2[:, :ncols],
                    lhsT=brow_sb[:],
                    rhs=ones_sb[:, goff:goff + ncols],
                    start=False, stop=True,
                )
                nc.vector.tensor_tensor(
                    out=h3_sb[:, goff:goff + ncols],
                    in0=ps2[:, :ncols],
                    in1=xres_sb[:, goff:goff + ncols],
                    op=bass.mybir.AluOpType.add,
                )
                # BN stats for this group
                sq_scr = sqp.tile([D, GTILES * TILE], f32, tag="sq")
                nc.scalar.activation(
                    sq_scr[:, :ncols],
                    h3_sb[:, goff:goff + ncols],
                    bass.mybir.ActivationFunctionType.Square,
                    accum_out=stat_q[:, g:g + 1],
                )
                nc.vector.reduce_sum(
                    stat_s[:, g:g + 1],
                    h3_sb[:, goff:goff + ncols],
                    axis=bass.mybir.AxisListType.X,
                )

            stats2 = constp.tile([D, 2], f32)
            nc.vector.reduce_sum(stats2[:, 0:1], stat_s[:],
                                 axis=bass.mybir.AxisListType.X)
            nc.vector.reduce_sum(stats2[:, 1:2], stat_q[:],
                                 axis=bass.mybir.AxisListType.X)

            statsg = constp.tile([D, 2], f32)
            if replica_groups is not None:
                nc.gpsimd.dma_start(cc_in[:], stats2[:])
                nc.gpsimd.collective_compute(
                    "AllReduce",
                    bass.mybir.AluOpType.add,
                    replica_groups=replica_groups,
                    ins=[cc_in[:]],
                    outs=[cc_out[:]],
                )
                nc.gpsimd.dma_start(statsg[:], cc_out[:])
            else:
                nc.vector.tensor_copy(statsg[:], stats2[:])

            # finalize BN constants: A = gamma / sqrt(var + eps), Bc = beta - mean*A
            eps_sb = constp.tile([D, 1], f32)
            nc.gpsimd.memset(eps_sb[:], float(eps))
            mean = constp.tile([D, 1], f32)
            esq = constp.tile([D, 1], f32)
            var = constp.tile([D, 1], f32)
            sd = constp.tile([D, 1], f32)
            rsd = constp.tile([D, 1], f32)
            A = constp.tile([D, 1], f32)
            Bc = constp.tile([D, 1], f32)
            inv_n = 1.0 / float(N)
            nc.vector.tensor_scalar_mul(mean[:], statsg[:, 0:1], inv_n)
            nc.vector.tensor_scalar_mul(esq[:], statsg[:, 1:2], inv_n)
            nc.vector.tensor_tensor(out=var[:], in0=mean[:], in1=mean[:],
                                    op=bass.mybir.AluOpType.mult)
            nc.vector.tensor_tensor(out=var[:], in0=esq[:], in1=var[:],
                                    op=bass.mybir.AluOpType.subtract)
            nc.scalar.activation(sd[:], var[:],
                                 bass.mybir.ActivationFunctionType.Sqrt,
                                 bias=eps_sb[:, 0:1], scale=1.0)
            nc.vector.reciprocal(rsd[:], sd[:])
            nc.vector.tensor_tensor(out=A[:], in0=rsd[:], in1=gam_sb[:],
                                    op=bass.mybir.AluOpType.mult)
            nc.vector.tensor_tensor(out=Bc[:], in0=mean[:], in1=A[:],
                                    op=bass.mybir.AluOpType.mult)
            nc.vector.tensor_tensor(out=Bc[:], in0=bet_sb[:], in1=Bc[:],
                                    op=bass.mybir.AluOpType.subtract)

            # apply BN + ReLU in place, then store
            nc.scalar.activation(h3_sb[:], h3_sb[:],
                                 bass.mybir.ActivationFunctionType.Relu,
                                 bias=Bc[:, 0:1], scale=A[:, 0:1])
            nc.sync.dma_start(out_d[:], h3_sb[:])

    # Raw Bass (Tile) skips Bacc's library/ISA lowering passes; without them
    # the extended instructions (DMAGatherAnt) have empty .instr bytes and
    # walrus fails with "ISA wrong length", and no LOAD_LIB is emitted.
    import bass_rust as _bass_rust
    from concourse.library_config import all_libraries, standard
    inst_type_to_lib_mask = {}
    for lib in all_libraries:
        for inst_type in lib.instructions:
            inst_type_to_lib_mask[inst_type] = inst_type_to_lib_mask.get(
                inst_type, 0) | (1 << lib.index)
    _bass_rust.insert_library_loads(
        nc, inst_type_to_lib_mask, len(all_libraries), standard.index)
    mybir.codegen_inst_isa_subclasses(nc)
    return nc


def make_in_maps(meta, per_core, waug, x_pad, gamma, beta, n_cores):
    maps = []
    for c in range(n_cores):
        pc = per_core[c]
        maps.append({
            "x_pad": x_pad,
            "idx16": pc["idx16"],
            "val": pc["val"],
            "m8": pc["m8"],
            "xres": pc["xres"],
            "ones": pc["ones"],
            "waug": waug,
            "gam": np.asarray(gamma, dtype=np.float32).reshape(D, 1),
            "bet": np.asarray(beta, dtype=np.float32).reshape(D, 1),
        })
    return maps


# ======================================================================
# entry point
# ======================================================================
_CACHE = {}

EPS = 1e-5
N_CORES = 8


def kernel(x, adj_val, W, b, gamma, beta, adj_row, adj_col):
    install()
    x = np.asarray(x); adj_val = np.asarray(adj_val)
    W = np.asarray(W); b = np.asarray(b)
    gamma = np.asarray(gamma); beta = np.asarray(beta)
    adj_row = np.asarray(adj_row).astype(np.int64)
    adj_col = np.asarray(adj_col).astype(np.int64)

    meta, per_core, waug, x_pad = host_prep(
        x, adj_val, adj_row, adj_col, W, b, N_CORES)
    in_maps = make_in_maps(meta, per_core, waug, x_pad, gamma, beta, N_CORES)

    key = (meta["B"], tuple(tuple(v) for v in meta["nb_shared"]))
    if key not in _CACHE:
        nc = build_nc(meta, N_CORES, EPS,
                      replica_groups=[list(range(N_CORES))])
        _CACHE[key] = nc
    nc = _CACHE[key]

    from concourse.bass_utils import run_bass_kernel_spmd
    res = run_bass_kernel_spmd(nc, in_maps, list(range(N_CORES)))
    out = host_post([res.results[c]["outp"] for c in range(N_CORES)],
                    dict(meta=meta, per_core=per_core), N_CORES)
    return out.astype(np.float32)


# revision 10
# speedup vs baseline: 1.0250x; 1.0250x over previous
"""Self-contained Trainium2 Bass kernel for the DecoConv GNN layer.

kernel(**inputs) takes the full (unsharded) numpy inputs and returns the full
[100000, 64] fp32 output. Internally: shards destination nodes across the 8
NeuronCores, builds + compiles one SPMD Bass/Tile program on first call, and
runs it via concourse's PJRT path on cores 0-7.
"""
import sys
if '/opt/trn_rl_repo' not in sys.path:
    sys.path.insert(0, '/opt/trn_rl_repo')

import numpy as np

# ======================================================================
# environment fixups (walrus single-sync-wait limit, NTFF hook, uploads)
# ======================================================================
"""Split multi-wait instructions in BIR JSON: this container's walrus supports
only ONE sync wait per instruction. Extra waits are moved onto standalone
EventSemaphore instructions inserted immediately before (same engine, in-order)."""
import orjson

# opcodes that must stay glued to the following instruction (weights load + matmul)
_GLUE_PREV = {"TensorLoad", "LoadStationary", "TensorLoadWeights", "LdWeights"}

def split_multiwaits_json(bir_bytes: bytes) -> bytes:
    d = orjson.loads(bir_bytes)
    n_split = 0
    uid = [0]
    for fn in d.get("functions", []):
        for blk in fn.get("blocks", []):
            insts = blk.get("instructions", [])
            out = []
            for inst in insts:
                si = inst.get("sync_info") or {}
                waits = si.get("on_wait") or []
                if len(waits) > 1:
                    n_split += 1
                    pre = []
                    for w in waits:
                        uid[0] += 1
                        pre.append({
                            "debug": inst.get("debug", 0),
                            "engine": inst["engine"],
                            "ins": [],
                            "name": f"{inst['name']}_sw{uid[0]}",
                            "opcode": "EventSemaphore",
                            "outs": [],
                            "sync_info": {"on_update": [], "on_wait": [w]},
                        })
                    si["on_wait"] = []
                    inst["sync_info"] = si
                    # insert before a glued weights-load if present
                    ip = len(out)
                    while ip > 0 and out[ip-1].get("opcode") in _GLUE_PREV and out[ip-1].get("engine") == inst["engine"]:
                        ip -= 1
                    out[ip:ip] = pre
                out.append(inst)
            blk["instructions"] = out
    return orjson.dumps(d), n_split

_installed = False

def _make_ntff_hook(so_path="/opt/axon/libaxon_pjrt.so"):
    import contextlib, ctypes
    lib = ctypes.CDLL(so_path)
    if not hasattr(lib, "axon_start_nrt_profile"):
        return None
    lib.axon_start_nrt_profile.argtypes = [ctypes.POINTER(ctypes.c_int64), ctypes.c_size_t]
    lib.axon_start_nrt_profile.restype = ctypes.c_int64
    lib.axon_stop_nrt_profile.argtypes = [ctypes.c_char_p]
    lib.axon_stop_nrt_profile.restype = ctypes.c_int64

    @contextlib.contextmanager
    def _hook(output_dir, device_ids):
        import jax
        jax.devices()
        if device_ids:
            ids = (ctypes.c_int64 * len(device_ids))(*device_ids)
            rc = lib.axon_start_nrt_profile(ids, len(device_ids))
        else:
            rc = lib.axon_start_nrt_profile(None, 0)
        if rc != 0:
            raise RuntimeError(f"axon_start_nrt_profile rc={rc}")
        try:
            yield
        finally:
            n = lib.axon_stop_nrt_profile(str(output_dir).encode())
            if n < 0:
                raise RuntimeError(f"axon_stop_nrt_profile rc={n}")
    return _hook


def install():
    global _installed
    if _installed:
        return
    from concourse import bass2jax, bass_utils
    orig = bass_utils.compile_bir_kernel
    def patched(ant_bir_str, compile_dir_path, neff_name, **kw):
        fixed, n = split_multiwaits_json(ant_bir_str if isinstance(ant_bir_str, bytes) else ant_bir_str.encode())
        return orig(fixed, compile_dir_path, neff_name=neff_name, **kw)
    bass2jax.compile_bir_kernel = patched

    # antenv.axon_hooks shim so run_bass_kernel_spmd(trace=True) works
    import sys, types
    try:
        import antenv.axon_hooks  # noqa
    except ImportError:
        hook = _make_ntff_hook()
        mod = types.ModuleType("antenv.axon_hooks")
        mod.get_axon_ntff_profile_hook = lambda: hook
        mod.set_axon_ntff_profile_hook = lambda h: None
        sys.modules["antenv.axon_hooks"] = mod
        import antenv
        antenv.axon_hooks = mod

    # no-op the artifact upload (no bucket access in this sandbox)
    bass_utils.upload_artifacts = lambda tmpdir: f"local:{tmpdir}"
    _installed = True


# ======================================================================
# kernel build + host pre/post processing
# ======================================================================
"""GNN message-passing kernel for TRN2 (dest-sharded SpMM + Linear + residual + BN + ReLU).

Layout strategy (v2):
- 784 global dest tiles of 128 rows (incl 2 empty pad tiles), snake-dealt to
  the 8 cores by edge count (98 tiles each) so per-position block counts are
  balanced; one shared SPMD program (per-position counts = max across cores).
- Tiles processed in groups of G=4 (25 groups). Per (group, bucket-of-25000
  source rows) one merged dma_gather (queue = bucket -> own Q7 core pair)
  fetches the section's edge slots (128B bf16 features in a 256B slot).
- DVE builds the one-hot M[e, i] = (r_e == i) per section against a dense
  materialized iota (step-1 operand first for 2x mode) and writes
  y = val * x_e into the padded half of each gathered slot.
- TensorE accumulates h1^T[d, i] += y_k^T @ M_k per dest tile in PSUM
  (features on partitions). Linear per group is a single N=512 matmul with a
  bias ones-row; residual add + BN stats (free-dim reductions + tiny
  AllReduce) and the fused scale/shift/ReLU run on DVE/ACT; output is bf16
  (host converts to fp32).
"""

import numpy as np
import ml_dtypes

BF16 = ml_dtypes.bfloat16
FP8 = ml_dtypes.float8_e4m3
D = 64
TILE = 128
XROW = 128          # padded bf16 row length of x in HBM (256 bytes)
BK = 25000          # source-bucket rows (int16 index range)
NBUCK = 4
GTILES = 4          # tiles per group (gather granularity; linear N=512)
N_GLOBAL_TILES = 784  # 782 real (100000/128 rounded up) + 2 pad


# ---------------------------------------------------------------- host prep

def host_prep(x, adj_val, adj_row, adj_col, W, b, n_cores):
    N = x.shape[0]
    assert N == 100000 and n_cores == 8
    n_tiles = N_GLOBAL_TILES // n_cores          # 98 per core
    S_pad = n_tiles * TILE                        # 12544
    n_groups = (n_tiles + GTILES - 1) // GTILES   # 25 (24x4 + 1x2)

    adj_row = np.asarray(adj_row)
    adj_col = np.asarray(adj_col)
    adj_val = np.asarray(adj_val)

    gt = adj_row // TILE                          # global tile of each edge
    cnt_g = np.bincount(gt, minlength=N_GLOBAL_TILES)

    # snake-deal global tiles (desc by count) to cores
    order_g = np.argsort(-cnt_g, kind="stable")
    core_tiles = np.empty((n_cores, n_tiles), dtype=np.int64)
    for p in range(n_tiles):
        blockk = order_g[p * n_cores:(p + 1) * n_cores]
        if p % 2 == 0:
            core_tiles[:, p] = blockk
        else:
            core_tiles[:, p] = blockk[::-1]
    core_of_tile = np.empty(N_GLOBAL_TILES, dtype=np.int64)
    pos_of_tile = np.empty(N_GLOBAL_TILES, dtype=np.int64)
    for c in range(n_cores):
        core_of_tile[core_tiles[c]] = c
        pos_of_tile[core_tiles[c]] = np.arange(n_tiles)

    ecore = core_of_tile[gt]
    epos = pos_of_tile[gt]
    ebuck = adj_col // BK

    # per (core, pos, bucket) counts -> shared block structure
    cnt3 = np.zeros((n_cores, n_tiles, NBUCK), dtype=np.int64)
    np.add.at(cnt3, (ecore, epos, ebuck), 1)
    nb_shared = (cnt3.max(0) + TILE - 1) // TILE          # [n_tiles, NBUCK]
    empty = nb_shared.sum(1) == 0
    nb_shared[empty, 0] = 1

    # block bases: group-major, bucket-major inside group, pos-major inside bucket
    block_base = np.zeros((n_tiles, NBUCK), dtype=np.int64)
    group_of_pos = np.arange(n_tiles) // GTILES
    B = 0
    grp_meta = []   # per group: dict(sect_nb[q], sect_base[q], tiles(pos list), blk_base)
    for g in range(n_groups):
        poss = [p for p in range(n_tiles) if group_of_pos[p] == g]
        gbase = B
        sect_nb = []
        sect_base = []
        for q in range(NBUCK):
            sect_base.append(B)
            for p in poss:
                block_base[p, q] = B
                B += nb_shared[p, q]
            sect_nb.append(B - sect_base[-1])
        grp_meta.append(dict(poss=poss, sect_nb=sect_nb, sect_base=sect_base,
                             blk_base=gbase, nblocks=B - gbase))
    max_grp_blocks = max(m["nblocks"] for m in grp_meta)
    max_sect_nb = max(max(m["sect_nb"]) for m in grp_meta)

    # slot assignment per core
    per_core = []
    for c in range(n_cores):
        m = ecore == c
        ep = epos[m]; eq = ebuck[m]
        ev = adj_val[m]; er = adj_row[m]; ec = adj_col[m]
        key = ep * NBUCK + eq
        sidx = np.argsort(key, kind="stable")
        ep = ep[sidx]; eq = eq[sidx]; ev = ev[sidx]; er = er[sidx]; ec = ec[sidx]
        kk = key[sidx]
        cnt_k = np.bincount(kk, minlength=n_tiles * NBUCK)
        start = np.zeros(n_tiles * NBUCK, dtype=np.int64)
        start[1:] = np.cumsum(cnt_k)[:-1]
        rank = np.arange(len(kk)) - start[kk]
        blk = block_base[ep, eq] + rank // TILE
        lane = rank % TILE

        val_arr = np.zeros((TILE, B), dtype=BF16)
        m8 = np.zeros((TILE, B * TILE), dtype=FP8)
        idx16 = np.zeros((TILE, B * 8), dtype=np.int16)

        val_arr[lane, blk] = ev.astype(BF16)
        rloc = (er - core_tiles[c][ep] * TILE).astype(np.int64)
        m8[lane, blk * TILE + rloc] = FP8(1.0)
        rel = (ec - eq * BK).astype(np.int16)
        colpos = blk * 8 + lane // 16
        rowpos = lane % 16
        for gg in range(8):
            idx16[rowpos + 16 * gg, colpos] = rel

        # transposed residual input + masked ones row (position order)
        xres = np.zeros((D, S_pad), dtype=BF16)
        ones_row = np.zeros((1, S_pad), dtype=BF16)
        for p in range(n_tiles):
            t = core_tiles[c][p]
            lo = t * TILE
            hi = min(lo + TILE, N)
            nvalid = max(0, hi - lo)
            if nvalid > 0:
                xres[:, p * TILE:p * TILE + nvalid] = \
                    np.asarray(x[lo:hi]).T.astype(BF16)
                ones_row[0, p * TILE:p * TILE + nvalid] = 1.0

        per_core.append(dict(val=val_arr, m8=m8, idx16=idx16, xres=xres,
                             ones=ones_row))

    waug = np.zeros((D + 1, D), dtype=BF16)
    waug[:D] = np.asarray(W, dtype=np.float32).T
    waug[D] = np.asarray(b, dtype=np.float32)
    x_pad = np.zeros((N, XROW), dtype=BF16)
    x_pad[:, :D] = np.asarray(x).astype(BF16)

    meta = dict(N=N, n_tiles=n_tiles, S_pad=S_pad, B=B, n_groups=n_groups,
                nb_shared=nb_shared.tolist(),
                block_base=block_base.tolist(),
                grp_meta=grp_meta,
                max_grp_blocks=max_grp_blocks,
                max_sect_nb=max_sect_nb,
                core_tiles=core_tiles.tolist())
    return meta, per_core, waug, x_pad


def host_post(results, metas, n_cores):
    """Assemble full [N, 64] fp32 output from per-core transposed bf16 outputs."""
    meta = metas["meta"]
    n_tiles = meta["n_tiles"]
    N = meta["N"]
    core_tiles = np.asarray(meta["core_tiles"])
    out = np.empty((N, D), dtype=np.float32)
    for c in range(n_cores):
        dev = np.asarray(results[c])  # [64, S_pad] bf16
        for p in range(n_tiles):
            t = core_tiles[c][p]
            lo = t * TILE
            hi = min(lo + TILE, N)
            if hi > lo:
                out[lo:hi] = dev[:, p * TILE:p * TILE + (hi - lo)].T.astype(np.float32)
    return out


# ---------------------------------------------------------------- device build

def build_nc(meta, n_cores, eps, replica_groups=None):
    from concourse import bass, mybir, tile

    N = meta["N"]
    S_pad = meta["S_pad"]
    n_tiles = meta["n_tiles"]
    B = meta["B"]
    nb_shared = meta["nb_shared"]
    block_base = meta["block_base"]
    grp_meta = meta["grp_meta"]
    n_groups = meta["n_groups"]
    max_grp_blocks = meta["max_grp_blocks"]
    max_sect_nb = meta["max_sect_nb"]
    f32 = mybir.dt.float32
    bf16 = mybir.dt.bfloat16
    i16 = mybir.dt.int16
    i32 = mybir.dt.int32

    nc = bass.Bass(debug=False, num_swdge_queues=4)
    x_d = nc.declare_dram_parameter("x_pad", [N, XROW], bf16, isOutput=False)
    idx_d = nc.declare_dram_parameter("idx16", [TILE, B * 8], i16, isOutput=False)
    val_d = nc.declare_dram_parameter("val", [TILE, B], bf16, isOutput=False)
    m8_d = nc.declare_dram_parameter("m8", [TILE, B * TILE], mybir.dt.float8e4, isOutput=False)
    xres_d = nc.declare_dram_parameter("xres", [D, S_pad], bf16, isOutput=False)
    ones_d = nc.declare_dram_parameter("ones", [1, S_pad], bf16, isOutput=False)
    waug_d = nc.declare_dram_parameter("waug", [D + 1, D], bf16, isOutput=False)
    gam_d = nc.declare_dram_parameter("gam", [D, 1], f32, isOutput=False)
    bet_d = nc.declare_dram_parameter("bet", [D, 1], f32, isOutput=False)
    out_d = nc.declare_dram_parameter("outp", [D, S_pad], bf16, isOutput=True)

    cc_in = nc.dram_tensor("cc_in", [D, 2], f32)
    cc_out = nc.dram_tensor("cc_out", [D, 2], f32, addr_space="Shared")

    with tile.TileContext(nc) as tc:
        with (
            tc.tile_pool(name="const", bufs=1) as constp,
            tc.tile_pool(name="big", bufs=1) as bigp,
            tc.tile_pool(name="xgp", bufs=3) as xgp,
            tc.tile_pool(name="mmp", bufs=3) as mmp,
            tc.tile_pool(name="idxp", bufs=3) as idxp,
            tc.tile_pool(name="h1p", bufs=2) as h1p,
            tc.tile_pool(name="psA", bufs=4, space="PSUM") as psA,
            tc.tile_pool(name="psB", bufs=2, space="PSUM") as psB,
        ):
            val_sb = bigp.tile([TILE, B], bf16)
            xres_sb = bigp.tile([D, S_pad], bf16)
            h3_sb = bigp.tile([D, S_pad], bf16)
            ones_sb = constp.tile([1, S_pad], bf16)
            waug_sb = constp.tile([D + 1, D], bf16)
            gam_sb = constp.tile([D, 1], f32)
            bet_sb = constp.tile([D, 1], f32)
            stat_s = constp.tile([D, n_groups], f32)
            stat_q = constp.tile([D, n_groups], f32)

            nc.sync.dma_start(val_sb[:], val_d[:])
            nc.sync.dma_start(xres_sb[:], xres_d[:])
            nc.sync.dma_start(ones_sb[:], ones_d[:])
            nc.sync.dma_start(waug_sb[:], waug_d[:])
            nc.sync.dma_start(gam_sb[:], gam_d[:])
            nc.sync.dma_start(bet_sb[:], bet_d[:])


            # one register per distinct gather size
            nidx_regs = {}
            for g in range(n_groups):
                for q in range(NBUCK):
                    nbs = grp_meta[g]["sect_nb"][q]
                    if nbs and nbs * TILE not in nidx_regs:
                        nidx_regs[nbs * TILE] = nc.gpsimd.to_reg(nbs * TILE)

            for g in range(n_groups):
                gm = grp_meta[g]
                poss = gm["poss"]
                gbase = gm["blk_base"]
                gnb = gm["nblocks"]
                ncols = len(poss) * TILE

                xg = xgp.tile([TILE, max_grp_blocks * XROW], bf16, tag="xg")
                mm = mmp.tile([TILE, max_grp_blocks * TILE], mybir.dt.float8e4, tag="mm")
                idx_sb = idxp.tile([TILE, max_grp_blocks * 8], i16, tag="idx")
                h1 = h1p.tile([D + 1, GTILES * TILE], bf16, tag="h1")

                nc.sync.dma_start(idx_sb[:, :gnb * 8],
                                  idx_d[:, gbase * 8:(gbase + gnb) * 8])
                nc.sync.dma_start(mm[:, :gnb * TILE],
                                  m8_d[:, gbase * TILE:(gbase + gnb) * TILE])

                for q in range(NBUCK):
                    nbs = gm["sect_nb"][q]
                    if nbs == 0:
                        continue
                    rel = gm["sect_base"][q] - gbase
                    nrow = min(BK, N - q * BK)
                    nc.gpsimd.dma_gather(
                        out_ap=xg[:, rel * XROW:(rel + nbs) * XROW].rearrange(
                            "p (b e) -> p b e", e=XROW),
                        in_ap=x_d[q * BK:q * BK + nrow, :],
                        idxs_ap=idx_sb[:, rel * 8:(rel + nbs) * 8],
                        num_idxs=nbs * TILE,
                        num_idxs_reg=nidx_regs[nbs * TILE],
                        elem_size=XROW,
                        elem_step=XROW,
                        queue_num=q,
                        single_packet=False,
                    )

                for q in range(NBUCK):
                    nbs = gm["sect_nb"][q]
                    if nbs == 0:
                        continue
                    rel = gm["sect_base"][q] - gbase
                    sbase = gm["sect_base"][q]
                    # y = val * x into the padded half of each slot
                    nc.vector.tensor_tensor(
                        out=xg[:, rel * XROW:(rel + nbs) * XROW].rearrange(
                            "p (b e) -> p b e", e=XROW)[:, :, D:2 * D],
                        in0=xg[:, rel * XROW:(rel + nbs) * XROW].rearrange(
                            "p (b e) -> p b e", e=XROW)[:, :, 0:D],
                        in1=val_sb[:, sbase:sbase + nbs].unsqueeze(2).to_broadcast(
                            [TILE, nbs, D]),
                        op=bass.mybir.AluOpType.mult,
                    )

                for j, p in enumerate(poss):
                    ps = psA.tile([D, TILE], f32, tag="ps")
                    blks = []
                    for q in range(NBUCK):
                        nb = nb_shared[p][q]
                        bb = block_base[p][q]
                        for k in range(nb):
                            blks.append(bb - gbase + k)
                    for ki, rb in enumerate(blks):
                        nc.tensor.matmul(
                            ps[:],
                            lhsT=xg[:, rb * XROW + D:rb * XROW + 2 * D],
                            rhs=mm[:, rb * TILE:(rb + 1) * TILE],
                            start=(ki == 0),
                            stop=(ki == len(blks) - 1),
                        )
                    nc.scalar.copy(h1[0:D, j * TILE:(j + 1) * TILE], ps[:])

                # bias ones row for this group's columns
                goff = poss[0] * TILE
                nc.vector.tensor_copy(h1[D:D + 1, :ncols],
                                      ones_sb[:, goff:goff + ncols])

                ps2 = psB.tile([D, GTILES * TILE], f32, tag="ps2")
                nc.tensor.matmul(
                    ps2[:, :ncols],
                    lhsT=waug_sb[:],
                    rhs=h1[:, :ncols],
                    start=True, stop=True,
                )
                nc.vector.tensor_tensor(
                    out=h3_sb[:, goff:goff + ncols],
                    in0=ps2[:, :ncols],
                    in1=xres_sb[:, goff:goff + ncols],
                    op=bass.mybir.AluOpType.add,
                )
                # BN stats for this group
                sq_scr = h1p.tile([D, GTILES * TILE], f32, tag="sq")
                nc.scalar.activation(
                    sq_scr[:, :ncols],
                    h3_sb[:, goff:goff + ncols],
                    bass.mybir.ActivationFunctionType.Square,
                    accum_out=stat_q[:, g:g + 1],
                )
                nc.vector.reduce_sum(
                    stat_s[:, g:g + 1],
                    h3_sb[:, goff:goff + ncols],
                    axis=bass.mybir.AxisListType.X,
                )

            stats2 = constp.tile([D, 2], f32)
            nc.vector.reduce_sum(stats2[:, 0:1], stat_s[:],
                                 axis=bass.mybir.AxisListType.X)
            nc.vector.reduce_sum(stats2[:, 1:2], stat_q[:],
                                 axis=bass.mybir.AxisListType.X)

            statsg = constp.tile([D, 2], f32)
            if replica_groups is not None:
                nc.gpsimd.dma_start(cc_in[:], stats2[:])
                nc.gpsimd.collective_compute(
                    "AllReduce",
                    bass.mybir.AluOpType.add,
                    replica_groups=replica_groups,
                    ins=[cc_in[:]],
                    outs=[cc_out[:]],
                )
                nc.gpsimd.dma_start(statsg[:], cc_out[:])
            else:
                nc.vector.tensor_copy(statsg[:], stats2[:])

            # finalize BN constants: A = gamma / sqrt(var + eps), Bc = beta - mean*A
            eps_sb = constp.tile([D, 1], f32)
            nc.gpsimd.memset(eps_sb[:], float(eps))
            mean = constp.tile([D, 1], f32)
            esq = constp.tile([D, 1], f32)
            var = constp.tile([D, 1], f32)
            sd = constp.tile([D, 1], f32)
            rsd = constp.tile([D, 1], f32)
            A = constp.tile([D, 1], f32)
            Bc = constp.tile([D, 1], f32)
            inv_n = 1.0 / float(N)
            nc.vector.tensor_scalar_mul(mean[:], statsg[:, 0:1], inv_n)
            nc.vector.tensor_scalar_mul(esq[:], statsg[:, 1:2], inv_n)
            nc.vector.tensor_tensor(out=var[:], in0=mean[:], in1=mean[:],
                                    op=bass.mybir.AluOpType.mult)
            nc.vector.tensor_tensor(out=var[:], in0=esq[:], in1=var[:],
                                    op=bass.mybir.AluOpType.subtract)
            nc.scalar.activation(sd[:], var[:],
                                 bass.mybir.ActivationFunctionType.Sqrt,
                                 bias=eps_sb[:, 0:1], scale=1.0)
            nc.vector.reciprocal(rsd[:], sd[:])
            nc.vector.tensor_tensor(out=A[:], in0=rsd[:], in1=gam_sb[:],
                                    op=bass.mybir.AluOpType.mult)
            nc.vector.tensor_tensor(out=Bc[:], in0=mean[:], in1=A[:],
                                    op=bass.mybir.AluOpType.mult)
            nc.vector.tensor_tensor(out=Bc[:], in0=bet_sb[:], in1=Bc[:],
                                    op=bass.mybir.AluOpType.subtract)

            # apply BN + ReLU in place, then store
            nc.scalar.activation(h3_sb[:], h3_sb[:],
                                 bass.mybir.ActivationFunctionType.Relu,
                                 bias=Bc[:, 0:1], scale=A[:, 0:1])
            nc.sync.dma_start(out_d[:], h3_sb[:])

    # Raw Bass (Tile) skips Bacc's library/ISA lowering passes; without them
    # the extended instructions (DMAGatherAnt) have empty .instr bytes and
    # walrus fails with "ISA wrong length", and no LOAD_LIB is emitted.
    import bass_rust as _bass_rust
    from concourse.library_config import all_libraries, standard
    inst_type_to_lib_mask = {}
    for lib in all_libraries:
        for inst_type in lib.instructions:
            inst_type_to_lib_mask[inst_type] = inst_type_to_lib_mask.get(
                inst_type, 0) | (1 << lib.index)
    _bass_rust.insert_library_loads(
        nc, inst_type_to_lib_mask, len(all_libraries), standard.index)
    mybir.codegen_inst_isa_subclasses(nc)
    return nc


def make_in_maps(meta, per_core, waug, x_pad, gamma, beta, n_cores):
    maps = []
    for c in range(n_cores):
        pc = per_core[c]
        maps.append({
            "x_pad": x_pad,
            "idx16": pc["idx16"],
            "val": pc["val"],
            "m8": pc["m8"],
            "xres": pc["xres"],
            "ones": pc["ones"],
            "waug": waug,
            "gam": np.asarray(gamma, dtype=np.float32).reshape(D, 1),
            "bet": np.asarray(beta, dtype=np.float32).reshape(D, 1),
        })
    return maps


# ======================================================================
# entry point
# ======================================================================
_CACHE = {}

EPS = 1e-5
N_CORES = 8


def kernel(x, adj_val, W, b, gamma, beta, adj_row, adj_col):
    install()
    x = np.asarray(x); adj_val = np.asarray(adj_val)
    W = np.asarray(W); b = np.asarray(b)
    gamma = np.asarray(gamma); beta = np.asarray(beta)
    adj_row = np.asarray(adj_row).astype(np.int64)
    adj_col = np.asarray(adj_col).astype(np.int64)

    meta, per_core, waug, x_pad = host_prep(
        x, adj_val, adj_row, adj_col, W, b, N_CORES)
    in_maps = make_in_maps(meta, per_core, waug, x_pad, gamma, beta, N_CORES)

    key = (meta["B"], tuple(tuple(v) for v in meta["nb_shared"]))
    if key not in _CACHE:
        nc = build_nc(meta, N_CORES, EPS,
                      replica_groups=[list(range(N_CORES))])
        _CACHE[key] = nc
    nc = _CACHE[key]

    from concourse.bass_utils import run_bass_kernel_spmd
    res = run_bass_kernel_spmd(nc, in_maps, list(range(N_CORES)))
    out = host_post([res.results[c]["outp"] for c in range(N_CORES)],
                    dict(meta=meta, per_core=per_core), N_CORES)
    return out.astype(np.float32)


# revision 11
# speedup vs baseline: 1.0938x; 1.0672x over previous
"""Self-contained Trainium2 Bass kernel for the DecoConv GNN layer.

kernel(**inputs) takes the full (unsharded) numpy inputs and returns the full
[100000, 64] fp32 output. Internally: shards destination nodes across the 8
NeuronCores, builds + compiles one SPMD Bass/Tile program on first call, and
runs it via concourse's PJRT path on cores 0-7.
"""
import sys
if '/opt/trn_rl_repo' not in sys.path:
    sys.path.insert(0, '/opt/trn_rl_repo')

import numpy as np

# ======================================================================
# environment fixups (walrus single-sync-wait limit, NTFF hook, uploads)
# ======================================================================
"""Split multi-wait instructions in BIR JSON: this container's walrus supports
only ONE sync wait per instruction. Extra waits are moved onto standalone
EventSemaphore instructions inserted immediately before (same engine, in-order)."""
import orjson

# opcodes that must stay glued to the following instruction (weights load + matmul)
_GLUE_PREV = {"TensorLoad", "LoadStationary", "TensorLoadWeights", "LdWeights"}

def split_multiwaits_json(bir_bytes: bytes) -> bytes:
    d = orjson.loads(bir_bytes)
    n_split = 0
    uid = [0]
    for fn in d.get("functions", []):
        for blk in fn.get("blocks", []):
            insts = blk.get("instructions", [])
            out = []
            for inst in insts:
                si = inst.get("sync_info") or {}
                waits = si.get("on_wait") or []
                if len(waits) > 1:
                    n_split += 1
                    pre = []
                    for w in waits:
                        uid[0] += 1
                        pre.append({
                            "debug": inst.get("debug", 0),
                            "engine": inst["engine"],
                            "ins": [],
                            "name": f"{inst['name']}_sw{uid[0]}",
                            "opcode": "EventSemaphore",
                            "outs": [],
                            "sync_info": {"on_update": [], "on_wait": [w]},
                        })
                    si["on_wait"] = []
                    inst["sync_info"] = si
                    # insert before a glued weights-load if present
                    ip = len(out)
                    while ip > 0 and out[ip-1].get("opcode") in _GLUE_PREV and out[ip-1].get("engine") == inst["engine"]:
                        ip -= 1
                    out[ip:ip] = pre
                out.append(inst)
            blk["instructions"] = out
    return orjson.dumps(d), n_split

_installed = False

def _make_ntff_hook(so_path="/opt/axon/libaxon_pjrt.so"):
    import contextlib, ctypes
    lib = ctypes.CDLL(so_path)
    if not hasattr(lib, "axon_start_nrt_profile"):
        return None
    lib.axon_start_nrt_profile.argtypes = [ctypes.POINTER(ctypes.c_int64), ctypes.c_size_t]
    lib.axon_start_nrt_profile.restype = ctypes.c_int64
    lib.axon_stop_nrt_profile.argtypes = [ctypes.c_char_p]
    lib.axon_stop_nrt_profile.restype = ctypes.c_int64

    @contextlib.contextmanager
    def _hook(output_dir, device_ids):
        import jax
        jax.devices()
        if device_ids:
            ids = (ctypes.c_int64 * len(device_ids))(*device_ids)
            rc = lib.axon_start_nrt_profile(ids, len(device_ids))
        else:
            rc = lib.axon_start_nrt_profile(None, 0)
        if rc != 0:
            raise RuntimeError(f"axon_start_nrt_profile rc={rc}")
        try:
            yield
        finally:
            n = lib.axon_stop_nrt_profile(str(output_dir).encode())
            if n < 0:
                raise RuntimeError(f"axon_stop_nrt_profile rc={n}")
    return _hook


def install():
    global _installed
    if _installed:
        return
    from concourse import bass2jax, bass_utils
    orig = bass_utils.compile_bir_kernel
    def patched(ant_bir_str, compile_dir_path, neff_name, **kw):
        fixed, n = split_multiwaits_json(ant_bir_str if isinstance(ant_bir_str, bytes) else ant_bir_str.encode())
        return orig(fixed, compile_dir_path, neff_name=neff_name, **kw)
    bass2jax.compile_bir_kernel = patched

    # antenv.axon_hooks shim so run_bass_kernel_spmd(trace=True) works
    import sys, types
    try:
        import antenv.axon_hooks  # noqa
    except ImportError:
        hook = _make_ntff_hook()
        mod = types.ModuleType("antenv.axon_hooks")
        mod.get_axon_ntff_profile_hook = lambda: hook
        mod.set_axon_ntff_profile_hook = lambda h: None
        sys.modules["antenv.axon_hooks"] = mod
        import antenv
        antenv.axon_hooks = mod

    # no-op the artifact upload (no bucket access in this sandbox)
    bass_utils.upload_artifacts = lambda tmpdir: f"local:{tmpdir}"
    _installed = True


# ======================================================================
# kernel build + host pre/post processing
# ======================================================================
"""GNN message-passing kernel for TRN2 (dest-sharded SpMM + Linear + residual + BN + ReLU).

Layout strategy (v2):
- 784 global dest tiles of 128 rows (incl 2 empty pad tiles), snake-dealt to
  the 8 cores by edge count (98 tiles each) so per-position block counts are
  balanced; one shared SPMD program (per-position counts = max across cores).
- Tiles processed in groups of G=4 (25 groups). Per (group, bucket-of-25000
  source rows) one merged dma_gather (queue = bucket -> own Q7 core pair)
  fetches the section's edge slots (128B bf16 features in a 256B slot).
- DVE builds the one-hot M[e, i] = (r_e == i) per section against a dense
  materialized iota (step-1 operand first for 2x mode) and writes
  y = val * x_e into the padded half of each gathered slot.
- TensorE accumulates h1^T[d, i] += y_k^T @ M_k per dest tile in PSUM
  (features on partitions). Linear per group is a single N=512 matmul with a
  bias ones-row; residual add + BN stats (free-dim reductions + tiny
  AllReduce) and the fused scale/shift/ReLU run on DVE/ACT; output is bf16
  (host converts to fp32).
"""

import numpy as np
import ml_dtypes

BF16 = ml_dtypes.bfloat16
FP8 = ml_dtypes.float8_e4m3
D = 64
TILE = 128
XROW = 128          # padded bf16 row length of x in HBM (256 bytes)
BK = 25000          # source-bucket rows (int16 index range)
NBUCK = 4
GTILES = 4          # tiles per group (gather granularity; linear N=512)
N_GLOBAL_TILES = 784  # 782 real (100000/128 rounded up) + 2 pad


# ---------------------------------------------------------------- host prep

def host_prep(x, adj_val, adj_row, adj_col, W, b, n_cores):
    N = x.shape[0]
    assert N == 100000 and n_cores == 8
    n_tiles = N_GLOBAL_TILES // n_cores          # 98 per core
    S_pad = n_tiles * TILE                        # 12544
    n_groups = (n_tiles + GTILES - 1) // GTILES   # 25 (24x4 + 1x2)

    adj_row = np.asarray(adj_row)
    adj_col = np.asarray(adj_col)
    adj_val = np.asarray(adj_val)

    gt = adj_row // TILE                          # global tile of each edge
    cnt_g = np.bincount(gt, minlength=N_GLOBAL_TILES)

    # snake-deal global tiles (desc by count) to cores
    order_g = np.argsort(-cnt_g, kind="stable")
    core_tiles = np.empty((n_cores, n_tiles), dtype=np.int64)
    for p in range(n_tiles):
        blockk = order_g[p * n_cores:(p + 1) * n_cores]
        if p % 2 == 0:
            core_tiles[:, p] = blockk
        else:
            core_tiles[:, p] = blockk[::-1]
    core_of_tile = np.empty(N_GLOBAL_TILES, dtype=np.int64)
    pos_of_tile = np.empty(N_GLOBAL_TILES, dtype=np.int64)
    for c in range(n_cores):
        core_of_tile[core_tiles[c]] = c
        pos_of_tile[core_tiles[c]] = np.arange(n_tiles)

    ecore = core_of_tile[gt]
    epos = pos_of_tile[gt]
    ebuck = adj_col // BK

    # per (core, pos, bucket) counts -> shared block structure
    cnt3 = np.zeros((n_cores, n_tiles, NBUCK), dtype=np.int64)
    np.add.at(cnt3, (ecore, epos, ebuck), 1)
    nb_shared = (cnt3.max(0) + TILE - 1) // TILE          # [n_tiles, NBUCK]
    empty = nb_shared.sum(1) == 0
    nb_shared[empty, 0] = 1

    # block bases: group-major, bucket-major inside group, pos-major inside bucket
    block_base = np.zeros((n_tiles, NBUCK), dtype=np.int64)
    group_of_pos = np.arange(n_tiles) // GTILES
    B = 0
    grp_meta = []   # per group: dict(sect_nb[q], sect_base[q], tiles(pos list), blk_base)
    for g in range(n_groups):
        poss = [p for p in range(n_tiles) if group_of_pos[p] == g]
        gbase = B
        sect_nb = []
        sect_base = []
        for q in range(NBUCK):
            sect_base.append(B)
            for p in poss:
                block_base[p, q] = B
                B += nb_shared[p, q]
            sect_nb.append(B - sect_base[-1])
        grp_meta.append(dict(poss=poss, sect_nb=sect_nb, sect_base=sect_base,
                             blk_base=gbase, nblocks=B - gbase))
    max_grp_blocks = max(m["nblocks"] for m in grp_meta)
    max_sect_nb = max(max(m["sect_nb"]) for m in grp_meta)

    # slot assignment per core
    per_core = []
    for c in range(n_cores):
        m = ecore == c
        ep = epos[m]; eq = ebuck[m]
        ev = adj_val[m]; er = adj_row[m]; ec = adj_col[m]
        key = ep * NBUCK + eq
        sidx = np.argsort(key, kind="stable")
        ep = ep[sidx]; eq = eq[sidx]; ev = ev[sidx]; er = er[sidx]; ec = ec[sidx]
        kk = key[sidx]
        cnt_k = np.bincount(kk, minlength=n_tiles * NBUCK)
        start = np.zeros(n_tiles * NBUCK, dtype=np.int64)
        start[1:] = np.cumsum(cnt_k)[:-1]
        rank = np.arange(len(kk)) - start[kk]
        blk = block_base[ep, eq] + rank // TILE
        lane = rank % TILE

        val_arr = np.zeros((TILE, B), dtype=BF16)
        m8 = np.zeros((TILE, B * TILE), dtype=FP8)
        idx16 = np.zeros((TILE, B * 8), dtype=np.int16)

        val_arr[lane, blk] = ev.astype(BF16)
        rloc = (er - core_tiles[c][ep] * TILE).astype(np.int64)
        m8[lane, blk * TILE + rloc] = FP8(1.0)
        rel = (ec - eq * BK).astype(np.int16)
        colpos = blk * 8 + lane // 16
        rowpos = lane % 16
        for gg in range(8):
            idx16[rowpos + 16 * gg, colpos] = rel

        # transposed residual input + masked ones row (position order)
        xres = np.zeros((D, S_pad), dtype=BF16)
        ones_row = np.zeros((1, S_pad), dtype=BF16)
        for p in range(n_tiles):
            t = core_tiles[c][p]
            lo = t * TILE
            hi = min(lo + TILE, N)
            nvalid = max(0, hi - lo)
            if nvalid > 0:
                xres[:, p * TILE:p * TILE + nvalid] = \
                    np.asarray(x[lo:hi]).T.astype(BF16)
                ones_row[0, p * TILE:p * TILE + nvalid] = 1.0

        per_core.append(dict(val=val_arr, m8=m8, idx16=idx16, xres=xres,
                             ones=ones_row))

    waug = np.zeros((D + 1, D), dtype=BF16)
    waug[:D] = np.asarray(W, dtype=np.float32).T
    waug[D] = np.asarray(b, dtype=np.float32)
    x_pad = np.zeros((N, XROW), dtype=BF16)
    x_pad[:, :D] = np.asarray(x).astype(BF16)

    meta = dict(N=N, n_tiles=n_tiles, S_pad=S_pad, B=B, n_groups=n_groups,
                nb_shared=nb_shared.tolist(),
                block_base=block_base.tolist(),
                grp_meta=grp_meta,
                max_grp_blocks=max_grp_blocks,
                max_sect_nb=max_sect_nb,
                core_tiles=core_tiles.tolist())
    return meta, per_core, waug, x_pad


def host_post(results, metas, n_cores):
    """Assemble full [N, 64] fp32 output from per-core transposed bf16 outputs."""
    meta = metas["meta"]
    n_tiles = meta["n_tiles"]
    N = meta["N"]
    core_tiles = np.asarray(meta["core_tiles"])
    out = np.empty((N, D), dtype=np.float32)
    for c in range(n_cores):
        dev = np.asarray(results[c])  # [64, S_pad] bf16
        for p in range(n_tiles):
            t = core_tiles[c][p]
            lo = t * TILE
            hi = min(lo + TILE, N)
            if hi > lo:
                out[lo:hi] = dev[:, p * TILE:p * TILE + (hi - lo)].T.astype(np.float32)
    return out


# ---------------------------------------------------------------- device build

def build_nc(meta, n_cores, eps, replica_groups=None):
    from concourse import bass, mybir, tile

    N = meta["N"]
    S_pad = meta["S_pad"]
    n_tiles = meta["n_tiles"]
    B = meta["B"]
    nb_shared = meta["nb_shared"]
    block_base = meta["block_base"]
    grp_meta = meta["grp_meta"]
    n_groups = meta["n_groups"]
    max_grp_blocks = meta["max_grp_blocks"]
    max_sect_nb = meta["max_sect_nb"]
    f32 = mybir.dt.float32
    bf16 = mybir.dt.bfloat16
    i16 = mybir.dt.int16
    i32 = mybir.dt.int32

    nc = bass.Bass(debug=False, num_swdge_queues=4)
    x_d = nc.declare_dram_parameter("x_pad", [N, XROW], bf16, isOutput=False)
    idx_d = nc.declare_dram_parameter("idx16", [TILE, B * 8], i16, isOutput=False)
    val_d = nc.declare_dram_parameter("val", [TILE, B], bf16, isOutput=False)
    m8_d = nc.declare_dram_parameter("m8", [TILE, B * TILE], mybir.dt.float8e4, isOutput=False)
    xres_d = nc.declare_dram_parameter("xres", [D, S_pad], bf16, isOutput=False)
    ones_d = nc.declare_dram_parameter("ones", [1, S_pad], bf16, isOutput=False)
    waug_d = nc.declare_dram_parameter("waug", [D + 1, D], bf16, isOutput=False)
    gam_d = nc.declare_dram_parameter("gam", [D, 1], f32, isOutput=False)
    bet_d = nc.declare_dram_parameter("bet", [D, 1], f32, isOutput=False)
    out_d = nc.declare_dram_parameter("outp", [D, S_pad], bf16, isOutput=True)

    cc_in = nc.dram_tensor("cc_in", [D, 2], f32)
    cc_out = nc.dram_tensor("cc_out", [D, 2], f32, addr_space="Shared")

    with tile.TileContext(nc) as tc:
        with (
            tc.tile_pool(name="const", bufs=1) as constp,
            tc.tile_pool(name="big", bufs=1) as bigp,
            tc.tile_pool(name="xgp", bufs=3) as xgp,
            tc.tile_pool(name="mmp", bufs=3) as mmp,
            tc.tile_pool(name="idxp", bufs=3) as idxp,
            tc.tile_pool(name="h1p", bufs=2) as h1p,
            tc.tile_pool(name="psA", bufs=4, space="PSUM") as psA,
            tc.tile_pool(name="psB", bufs=2, space="PSUM") as psB,
        ):
            val_sb = bigp.tile([TILE, B], bf16)
            xres_sb = bigp.tile([D, S_pad], bf16)
            h3_sb = bigp.tile([D, S_pad], bf16)
            waug_sb = constp.tile([D + 1, D], bf16)
            gam_sb = constp.tile([D, 1], f32)
            bet_sb = constp.tile([D, 1], f32)
            stat_s = constp.tile([D, n_groups], f32)
            stat_q = constp.tile([D, n_groups], f32)

            nc.scalar.dma_start(val_sb[:], val_d[:])
            nc.scalar.dma_start(xres_sb[:], xres_d[:])
            nc.scalar.dma_start(waug_sb[:], waug_d[:])
            nc.scalar.dma_start(gam_sb[:], gam_d[:])
            nc.scalar.dma_start(bet_sb[:], bet_d[:])


            # one register per distinct gather size
            nidx_regs = {}
            for g in range(n_groups):
                for q in range(NBUCK):
                    nbs = grp_meta[g]["sect_nb"][q]
                    if nbs and nbs * TILE not in nidx_regs:
                        nidx_regs[nbs * TILE] = nc.gpsimd.to_reg(nbs * TILE)

            for g in range(n_groups):
                gm = grp_meta[g]
                poss = gm["poss"]
                gbase = gm["blk_base"]
                gnb = gm["nblocks"]
                ncols = len(poss) * TILE

                xg = xgp.tile([TILE, max_grp_blocks * XROW], bf16, tag="xg")
                mm = mmp.tile([TILE, max_grp_blocks * TILE], mybir.dt.float8e4, tag="mm")
                idx_sb = idxp.tile([TILE, max_grp_blocks * 8], i16, tag="idx")
                h1 = h1p.tile([D + 1, GTILES * TILE], bf16, tag="h1")

                nc.sync.dma_start(idx_sb[:, :gnb * 8],
                                  idx_d[:, gbase * 8:(gbase + gnb) * 8])
                nc.sync.dma_start(mm[:, :gnb * TILE],
                                  m8_d[:, gbase * TILE:(gbase + gnb) * TILE])

                for q in range(NBUCK):
                    nbs = gm["sect_nb"][q]
                    if nbs == 0:
                        continue
                    rel = gm["sect_base"][q] - gbase
                    nrow = min(BK, N - q * BK)
                    nc.gpsimd.dma_gather(
                        out_ap=xg[:, rel * XROW:(rel + nbs) * XROW].rearrange(
                            "p (b e) -> p b e", e=XROW),
                        in_ap=x_d[q * BK:q * BK + nrow, :],
                        idxs_ap=idx_sb[:, rel * 8:(rel + nbs) * 8],
                        num_idxs=nbs * TILE,
                        num_idxs_reg=nidx_regs[nbs * TILE],
                        elem_size=XROW,
                        elem_step=XROW,
                        queue_num=q,
                        single_packet=False,
                    )

                for q in range(NBUCK):
                    nbs = gm["sect_nb"][q]
                    if nbs == 0:
                        continue
                    rel = gm["sect_base"][q] - gbase
                    sbase = gm["sect_base"][q]
                    # y = val * x into the padded half of each slot
                    nc.vector.tensor_tensor(
                        out=xg[:, rel * XROW:(rel + nbs) * XROW].rearrange(
                            "p (b e) -> p b e", e=XROW)[:, :, D:2 * D],
                        in0=xg[:, rel * XROW:(rel + nbs) * XROW].rearrange(
                            "p (b e) -> p b e", e=XROW)[:, :, 0:D],
                        in1=val_sb[:, sbase:sbase + nbs].unsqueeze(2).to_broadcast(
                            [TILE, nbs, D]),
                        op=bass.mybir.AluOpType.mult,
                    )

                for j, p in enumerate(poss):
                    ps = psA.tile([D, TILE], f32, tag="ps")
                    blks = []
                    for q in range(NBUCK):
                        nb = nb_shared[p][q]
                        bb = block_base[p][q]
                        for k in range(nb):
                            blks.append(bb - gbase + k)
                    for ki, rb in enumerate(blks):
                        nc.tensor.matmul(
                            ps[:],
                            lhsT=xg[:, rb * XROW + D:rb * XROW + 2 * D],
                            rhs=mm[:, rb * TILE:(rb + 1) * TILE],
                            start=(ki == 0),
                            stop=(ki == len(blks) - 1),
                        )
                    nc.scalar.copy(h1[0:D, j * TILE:(j + 1) * TILE], ps[:])

                # bias ones row for this group's columns (tiny DMA, off DVE)
                goff = poss[0] * TILE
                nc.scalar.dma_start(h1[D:D + 1, :ncols],
                                    ones_d[:, goff:goff + ncols])

                ps2 = psB.tile([D, GTILES * TILE], f32, tag="ps2")
                nc.tensor.matmul(
                    ps2[:, :ncols],
                    lhsT=waug_sb[:],
                    rhs=h1[:, :ncols],
                    start=True, stop=True,
                )
                nc.vector.tensor_tensor(
                    out=h3_sb[:, goff:goff + ncols],
                    in0=ps2[:, :ncols],
                    in1=xres_sb[:, goff:goff + ncols],
                    op=bass.mybir.AluOpType.add,
                )
                # BN stats for this group
                sq_scr = h1p.tile([D, GTILES * TILE], f32, tag="sq")
                nc.scalar.activation(
                    sq_scr[:, :ncols],
                    h3_sb[:, goff:goff + ncols],
                    bass.mybir.ActivationFunctionType.Square,
                    accum_out=stat_q[:, g:g + 1],
                )
                nc.vector.reduce_sum(
                    stat_s[:, g:g + 1],
                    h3_sb[:, goff:goff + ncols],
                    axis=bass.mybir.AxisListType.X,
                )

            stats2 = constp.tile([D, 2], f32)
            nc.vector.reduce_sum(stats2[:, 0:1], stat_s[:],
                                 axis=bass.mybir.AxisListType.X)
            nc.vector.reduce_sum(stats2[:, 1:2], stat_q[:],
                                 axis=bass.mybir.AxisListType.X)

            statsg = constp.tile([D, 2], f32)
            if replica_groups is not None:
                nc.gpsimd.dma_start(cc_in[:], stats2[:])
                nc.gpsimd.collective_compute(
                    "AllReduce",
                    bass.mybir.AluOpType.add,
                    replica_groups=replica_groups,
                    ins=[cc_in[:]],
                    outs=[cc_out[:]],
                )
                nc.gpsimd.dma_start(statsg[:], cc_out[:])
            else:
                nc.vector.tensor_copy(statsg[:], stats2[:])

            # finalize BN constants: A = gamma / sqrt(var + eps), Bc = beta - mean*A
            eps_sb = constp.tile([D, 1], f32)
            nc.gpsimd.memset(eps_sb[:], float(eps))
            mean = constp.tile([D, 1], f32)
            esq = constp.tile([D, 1], f32)
            var = constp.tile([D, 1], f32)
            sd = constp.tile([D, 1], f32)
            rsd = constp.tile([D, 1], f32)
            A = constp.tile([D, 1], f32)
            Bc = constp.tile([D, 1], f32)
            inv_n = 1.0 / float(N)
            nc.vector.tensor_scalar_mul(mean[:], statsg[:, 0:1], inv_n)
            nc.vector.tensor_scalar_mul(esq[:], statsg[:, 1:2], inv_n)
            nc.vector.tensor_tensor(out=var[:], in0=mean[:], in1=mean[:],
                                    op=bass.mybir.AluOpType.mult)
            nc.vector.tensor_tensor(out=var[:], in0=esq[:], in1=var[:],
                                    op=bass.mybir.AluOpType.subtract)
            nc.scalar.activation(sd[:], var[:],
                                 bass.mybir.ActivationFunctionType.Sqrt,
                                 bias=eps_sb[:, 0:1], scale=1.0)
            nc.vector.reciprocal(rsd[:], sd[:])
            nc.vector.tensor_tensor(out=A[:], in0=rsd[:], in1=gam_sb[:],
                                    op=bass.mybir.AluOpType.mult)
            nc.vector.tensor_tensor(out=Bc[:], in0=mean[:], in1=A[:],
                                    op=bass.mybir.AluOpType.mult)
            nc.vector.tensor_tensor(out=Bc[:], in0=bet_sb[:], in1=Bc[:],
                                    op=bass.mybir.AluOpType.subtract)

            # apply BN + ReLU in place, then store
            nc.scalar.activation(h3_sb[:], h3_sb[:],
                                 bass.mybir.ActivationFunctionType.Relu,
                                 bias=Bc[:, 0:1], scale=A[:, 0:1])
            nc.sync.dma_start(out_d[:], h3_sb[:])

    # Raw Bass (Tile) skips Bacc's library/ISA lowering passes; without them
    # the extended instructions (DMAGatherAnt) have empty .instr bytes and
    # walrus fails with "ISA wrong length", and no LOAD_LIB is emitted.
    import bass_rust as _bass_rust
    from concourse.library_config import all_libraries, standard
    inst_type_to_lib_mask = {}
    for lib in all_libraries:
        for inst_type in lib.instructions:
            inst_type_to_lib_mask[inst_type] = inst_type_to_lib_mask.get(
                inst_type, 0) | (1 << lib.index)
    _bass_rust.insert_library_loads(
        nc, inst_type_to_lib_mask, len(all_libraries), standard.index)
    mybir.codegen_inst_isa_subclasses(nc)
    return nc


def make_in_maps(meta, per_core, waug, x_pad, gamma, beta, n_cores):
    maps = []
    for c in range(n_cores):
        pc = per_core[c]
        maps.append({
            "x_pad": x_pad,
            "idx16": pc["idx16"],
            "val": pc["val"],
            "m8": pc["m8"],
            "xres": pc["xres"],
            "ones": pc["ones"],
            "waug": waug,
            "gam": np.asarray(gamma, dtype=np.float32).reshape(D, 1),
            "bet": np.asarray(beta, dtype=np.float32).reshape(D, 1),
        })
    return maps


# ======================================================================
# entry point
# ======================================================================
_CACHE = {}

EPS = 1e-5
N_CORES = 8


def kernel(x, adj_val, W, b, gamma, beta, adj_row, adj_col):
    install()
    x = np.asarray(x); adj_val = np.asarray(adj_val)
    W = np.asarray(W); b = np.asarray(b)
    gamma = np.asarray(gamma); beta = np.asarray(beta)
    adj_row = np.asarray(adj_row).astype(np.int64)
    adj_col = np.asarray(adj_col).astype(np.int64)

    meta, per_core, waug, x_pad = host_prep(
        x, adj_val, adj_row, adj_col, W, b, N_CORES)
    in_maps = make_in_maps(meta, per_core, waug, x_pad, gamma, beta, N_CORES)

    key = (meta["B"], tuple(tuple(v) for v in meta["nb_shared"]))
    if key not in _CACHE:
        nc = build_nc(meta, N_CORES, EPS,
                      replica_groups=[list(range(N_CORES))])
        _CACHE[key] = nc
    nc = _CACHE[key]

    from concourse.bass_utils import run_bass_kernel_spmd
    res = run_bass_kernel_spmd(nc, in_maps, list(range(N_CORES)))
    out = host_post([res.results[c]["outp"] for c in range(N_CORES)],
                    dict(meta=meta, per_core=per_core), N_CORES)
    return out.astype(np.float32)


# revision 13
# speedup vs baseline: 1.1994x; 1.0965x over previous
"""Self-contained Trainium2 Bass kernel for the DecoConv GNN layer.

kernel(**inputs) takes the full (unsharded) numpy inputs and returns the full
[100000, 64] fp32 output. Internally: shards destination nodes across the 8
NeuronCores, builds + compiles one SPMD Bass/Tile program on first call, and
runs it via concourse's PJRT path on cores 0-7.
"""
import sys
if '/opt/trn_rl_repo' not in sys.path:
    sys.path.insert(0, '/opt/trn_rl_repo')

import numpy as np

# ======================================================================
# environment fixups (walrus single-sync-wait limit, NTFF hook, uploads)
# ======================================================================
"""Split multi-wait instructions in BIR JSON: this container's walrus supports
only ONE sync wait per instruction. Extra waits are moved onto standalone
EventSemaphore instructions inserted immediately before (same engine, in-order)."""
import orjson

# opcodes that must stay glued to the following instruction (weights load + matmul)
_GLUE_PREV = {"TensorLoad", "LoadStationary", "TensorLoadWeights", "LdWeights"}

def split_multiwaits_json(bir_bytes: bytes) -> bytes:
    d = orjson.loads(bir_bytes)
    n_split = 0
    uid = [0]
    for fn in d.get("functions", []):
        for blk in fn.get("blocks", []):
            insts = blk.get("instructions", [])
            out = []
            for inst in insts:
                si = inst.get("sync_info") or {}
                waits = si.get("on_wait") or []
                if len(waits) > 1:
                    n_split += 1
                    pre = []
                    for w in waits:
                        uid[0] += 1
                        pre.append({
                            "debug": inst.get("debug", 0),
                            "engine": inst["engine"],
                            "ins": [],
                            "name": f"{inst['name']}_sw{uid[0]}",
                            "opcode": "EventSemaphore",
                            "outs": [],
                            "sync_info": {"on_update": [], "on_wait": [w]},
                        })
                    si["on_wait"] = []
                    inst["sync_info"] = si
                    # insert before a glued weights-load if present
                    ip = len(out)
                    while ip > 0 and out[ip-1].get("opcode") in _GLUE_PREV and out[ip-1].get("engine") == inst["engine"]:
                        ip -= 1
                    out[ip:ip] = pre
                out.append(inst)
            blk["instructions"] = out
    return orjson.dumps(d), n_split

_installed = False

def _make_ntff_hook(so_path="/opt/axon/libaxon_pjrt.so"):
    import contextlib, ctypes
    lib = ctypes.CDLL(so_path)
    if not hasattr(lib, "axon_start_nrt_profile"):
        return None
    lib.axon_start_nrt_profile.argtypes = [ctypes.POINTER(ctypes.c_int64), ctypes.c_size_t]
    lib.axon_start_nrt_profile.restype = ctypes.c_int64
    lib.axon_stop_nrt_profile.argtypes = [ctypes.c_char_p]
    lib.axon_stop_nrt_profile.restype = ctypes.c_int64

    @contextlib.contextmanager
    def _hook(output_dir, device_ids):
        import jax
        jax.devices()
        if device_ids:
            ids = (ctypes.c_int64 * len(device_ids))(*device_ids)
            rc = lib.axon_start_nrt_profile(ids, len(device_ids))
        else:
            rc = lib.axon_start_nrt_profile(None, 0)
        if rc != 0:
            raise RuntimeError(f"axon_start_nrt_profile rc={rc}")
        try:
            yield
        finally:
            n = lib.axon_stop_nrt_profile(str(output_dir).encode())
            if n < 0:
                raise RuntimeError(f"axon_stop_nrt_profile rc={n}")
    return _hook


def install():
    global _installed
    if _installed:
        return
    from concourse import bass2jax, bass_utils
    orig = bass_utils.compile_bir_kernel
    def patched(ant_bir_str, compile_dir_path, neff_name, **kw):
        fixed, n = split_multiwaits_json(ant_bir_str if isinstance(ant_bir_str, bytes) else ant_bir_str.encode())
        return orig(fixed, compile_dir_path, neff_name=neff_name, **kw)
    bass2jax.compile_bir_kernel = patched

    # antenv.axon_hooks shim so run_bass_kernel_spmd(trace=True) works
    import sys, types
    try:
        import antenv.axon_hooks  # noqa
    except ImportError:
        hook = _make_ntff_hook()
        mod = types.ModuleType("antenv.axon_hooks")
        mod.get_axon_ntff_profile_hook = lambda: hook
        mod.set_axon_ntff_profile_hook = lambda h: None
        sys.modules["antenv.axon_hooks"] = mod
        import antenv
        antenv.axon_hooks = mod

    # no-op the artifact upload (no bucket access in this sandbox)
    bass_utils.upload_artifacts = lambda tmpdir: f"local:{tmpdir}"
    _installed = True


# ======================================================================
# kernel build + host pre/post processing
# ======================================================================
"""GNN message-passing kernel for TRN2 (dest-sharded SpMM + Linear + residual + BN + ReLU).

Layout strategy (v2):
- 784 global dest tiles of 128 rows (incl 2 empty pad tiles), snake-dealt to
  the 8 cores by edge count (98 tiles each) so per-position block counts are
  balanced; one shared SPMD program (per-position counts = max across cores).
- Tiles processed in groups of G=4 (25 groups). Per (group, bucket-of-25000
  source rows) one merged dma_gather (queue = bucket -> own Q7 core pair)
  fetches the section's edge slots (128B bf16 features in a 256B slot).
- DVE builds the one-hot M[e, i] = (r_e == i) per section against a dense
  materialized iota (step-1 operand first for 2x mode) and writes
  y = val * x_e into the padded half of each gathered slot.
- TensorE accumulates h1^T[d, i] += y_k^T @ M_k per dest tile in PSUM
  (features on partitions). Linear per group is a single N=512 matmul with a
  bias ones-row; residual add + BN stats (free-dim reductions + tiny
  AllReduce) and the fused scale/shift/ReLU run on DVE/ACT; output is bf16
  (host converts to fp32).
"""

import numpy as np
import ml_dtypes

BF16 = ml_dtypes.bfloat16
FP8 = ml_dtypes.float8_e4m3
D = 64
TILE = 128
XROW = 128          # padded bf16 row length of x in HBM (256 bytes)
BK = 25000          # source-bucket rows (int16 index range)
NBUCK = 4
GTILES = 4          # tiles per group (gather granularity; linear N=512)
N_GLOBAL_TILES = 784  # 782 real (100000/128 rounded up) + 2 pad


# ---------------------------------------------------------------- host prep

def host_prep(x, adj_val, adj_row, adj_col, W, b, n_cores):
    N = x.shape[0]
    assert N == 100000 and n_cores == 8
    n_tiles = N_GLOBAL_TILES // n_cores          # 98 per core
    S_pad = n_tiles * TILE                        # 12544
    n_groups = (n_tiles + GTILES - 1) // GTILES   # 25 (24x4 + 1x2)

    adj_row = np.asarray(adj_row)
    adj_col = np.asarray(adj_col)
    adj_val = np.asarray(adj_val)

    gt = adj_row // TILE                          # global tile of each edge
    cnt_g = np.bincount(gt, minlength=N_GLOBAL_TILES)

    # snake-deal global tiles (desc by count) to cores
    order_g = np.argsort(-cnt_g, kind="stable")
    core_tiles = np.empty((n_cores, n_tiles), dtype=np.int64)
    for p in range(n_tiles):
        blockk = order_g[p * n_cores:(p + 1) * n_cores]
        if p % 2 == 0:
            core_tiles[:, p] = blockk
        else:
            core_tiles[:, p] = blockk[::-1]
    core_of_tile = np.empty(N_GLOBAL_TILES, dtype=np.int64)
    pos_of_tile = np.empty(N_GLOBAL_TILES, dtype=np.int64)
    for c in range(n_cores):
        core_of_tile[core_tiles[c]] = c
        pos_of_tile[core_tiles[c]] = np.arange(n_tiles)

    ecore = core_of_tile[gt]
    epos = pos_of_tile[gt]
    ebuck = adj_col // BK

    # per (core, pos, bucket) counts -> shared block structure
    cnt3 = np.zeros((n_cores, n_tiles, NBUCK), dtype=np.int64)
    np.add.at(cnt3, (ecore, epos, ebuck), 1)
    nb_shared = (cnt3.max(0) + TILE - 1) // TILE          # [n_tiles, NBUCK]
    empty = nb_shared.sum(1) == 0
    nb_shared[empty, 0] = 1

    # block bases: group-major, bucket-major inside group, pos-major inside bucket
    block_base = np.zeros((n_tiles, NBUCK), dtype=np.int64)
    group_of_pos = np.arange(n_tiles) // GTILES
    B = 0
    grp_meta = []   # per group: dict(sect_nb[q], sect_base[q], tiles(pos list), blk_base)
    for g in range(n_groups):
        poss = [p for p in range(n_tiles) if group_of_pos[p] == g]
        gbase = B
        sect_nb = []
        sect_base = []
        for q in range(NBUCK):
            sect_base.append(B)
            for p in poss:
                block_base[p, q] = B
                B += nb_shared[p, q]
            sect_nb.append(B - sect_base[-1])
        grp_meta.append(dict(poss=poss, sect_nb=sect_nb, sect_base=sect_base,
                             blk_base=gbase, nblocks=B - gbase))
    max_grp_blocks = max(m["nblocks"] for m in grp_meta)
    max_sect_nb = max(max(m["sect_nb"]) for m in grp_meta)

    # slot assignment per core
    per_core = []
    for c in range(n_cores):
        m = ecore == c
        ep = epos[m]; eq = ebuck[m]
        ev = adj_val[m]; er = adj_row[m]; ec = adj_col[m]
        key = ep * NBUCK + eq
        sidx = np.argsort(key, kind="stable")
        ep = ep[sidx]; eq = eq[sidx]; ev = ev[sidx]; er = er[sidx]; ec = ec[sidx]
        kk = key[sidx]
        cnt_k = np.bincount(kk, minlength=n_tiles * NBUCK)
        start = np.zeros(n_tiles * NBUCK, dtype=np.int64)
        start[1:] = np.cumsum(cnt_k)[:-1]
        rank = np.arange(len(kk)) - start[kk]
        blk = block_base[ep, eq] + rank // TILE
        lane = rank % TILE

        val_arr = np.zeros((TILE, B), dtype=BF16)
        m8 = np.zeros((TILE, B * TILE), dtype=FP8)
        idx16 = np.zeros((TILE, B * 8), dtype=np.int16)

        val_arr[lane, blk] = ev.astype(BF16)
        rloc = (er - core_tiles[c][ep] * TILE).astype(np.int64)
        m8[lane, blk * TILE + rloc] = FP8(1.0)
        rel = (ec - eq * BK).astype(np.int16)
        colpos = blk * 8 + lane // 16
        rowpos = lane % 16
        for gg in range(8):
            idx16[rowpos + 16 * gg, colpos] = rel

        # transposed residual input + masked ones row (position order)
        xres = np.zeros((D, S_pad), dtype=BF16)
        ones_row = np.zeros((1, S_pad), dtype=BF16)
        for p in range(n_tiles):
            t = core_tiles[c][p]
            lo = t * TILE
            hi = min(lo + TILE, N)
            nvalid = max(0, hi - lo)
            if nvalid > 0:
                xres[:, p * TILE:p * TILE + nvalid] = \
                    np.asarray(x[lo:hi]).T.astype(BF16)
                ones_row[0, p * TILE:p * TILE + nvalid] = 1.0

        per_core.append(dict(val=val_arr, m8=m8, idx16=idx16, xres=xres,
                             ones=ones_row))

    waug = np.zeros((D + 1, D), dtype=BF16)
    waug[:D] = np.asarray(W, dtype=np.float32).T
    waug[D] = np.asarray(b, dtype=np.float32)
    x_pad = np.zeros((N, XROW), dtype=BF16)
    x_pad[:, :D] = np.asarray(x).astype(BF16)

    meta = dict(N=N, n_tiles=n_tiles, S_pad=S_pad, B=B, n_groups=n_groups,
                nb_shared=nb_shared.tolist(),
                block_base=block_base.tolist(),
                grp_meta=grp_meta,
                max_grp_blocks=max_grp_blocks,
                max_sect_nb=max_sect_nb,
                core_tiles=core_tiles.tolist())
    return meta, per_core, waug, x_pad


def host_post(results, metas, n_cores):
    """Assemble full [N, 64] fp32 output from per-core transposed bf16 outputs."""
    meta = metas["meta"]
    n_tiles = meta["n_tiles"]
    N = meta["N"]
    core_tiles = np.asarray(meta["core_tiles"])
    out = np.empty((N, D), dtype=np.float32)
    for c in range(n_cores):
        dev = np.asarray(results[c])  # [64, S_pad] bf16
        for p in range(n_tiles):
            t = core_tiles[c][p]
            lo = t * TILE
            hi = min(lo + TILE, N)
            if hi > lo:
                out[lo:hi] = dev[:, p * TILE:p * TILE + (hi - lo)].T.astype(np.float32)
    return out


# ---------------------------------------------------------------- device build

def build_nc(meta, n_cores, eps, replica_groups=None):
    from concourse import bass, mybir, tile

    N = meta["N"]
    S_pad = meta["S_pad"]
    n_tiles = meta["n_tiles"]
    B = meta["B"]
    nb_shared = meta["nb_shared"]
    block_base = meta["block_base"]
    grp_meta = meta["grp_meta"]
    n_groups = meta["n_groups"]
    max_grp_blocks = meta["max_grp_blocks"]
    max_sect_nb = meta["max_sect_nb"]
    f32 = mybir.dt.float32
    bf16 = mybir.dt.bfloat16
    i16 = mybir.dt.int16
    i32 = mybir.dt.int32

    nc = bass.Bass(debug=False, num_swdge_queues=4)
    x_d = nc.declare_dram_parameter("x_pad", [N, XROW], bf16, isOutput=False)
    idx_d = nc.declare_dram_parameter("idx16", [TILE, B * 8], i16, isOutput=False)
    val_d = nc.declare_dram_parameter("val", [TILE, B], bf16, isOutput=False)
    m8_d = nc.declare_dram_parameter("m8", [TILE, B * TILE], mybir.dt.float8e4, isOutput=False)
    xres_d = nc.declare_dram_parameter("xres", [D, S_pad], bf16, isOutput=False)
    ones_d = nc.declare_dram_parameter("ones", [1, S_pad], bf16, isOutput=False)
    waug_d = nc.declare_dram_parameter("waug", [D + 1, D], bf16, isOutput=False)
    gam_d = nc.declare_dram_parameter("gam", [D, 1], f32, isOutput=False)
    bet_d = nc.declare_dram_parameter("bet", [D, 1], f32, isOutput=False)
    out_d = nc.declare_dram_parameter("outp", [D, S_pad], bf16, isOutput=True)

    cc_in = nc.dram_tensor("cc_in", [D, 2], f32)
    cc_out = nc.dram_tensor("cc_out", [D, 2], f32, addr_space="Shared")

    with tile.TileContext(nc) as tc:
        with (
            tc.tile_pool(name="const", bufs=1) as constp,
            tc.tile_pool(name="big", bufs=1) as bigp,
            tc.tile_pool(name="xgp", bufs=3) as xgp,
            tc.tile_pool(name="mmp", bufs=3) as mmp,
            tc.tile_pool(name="idxp", bufs=3) as idxp,
            tc.tile_pool(name="h1p", bufs=2) as h1p,
            tc.tile_pool(name="psA", bufs=6, space="PSUM") as psA,
            tc.tile_pool(name="psB", bufs=2, space="PSUM") as psB,
        ):
            val_sb = bigp.tile([TILE, B], bf16)
            xres_sb = bigp.tile([D, S_pad], bf16)
            h3_sb = bigp.tile([D, S_pad], bf16)
            waug_sb = constp.tile([D + 1, D], bf16)
            gam_sb = constp.tile([D, 1], f32)
            bet_sb = constp.tile([D, 1], f32)
            stat_s = constp.tile([D, n_groups], f32)
            stat_q = constp.tile([D, n_groups], f32)

            nc.scalar.dma_start(val_sb[:], val_d[:])
            nc.scalar.dma_start(xres_sb[:], xres_d[:])
            nc.scalar.dma_start(waug_sb[:], waug_d[:])
            nc.scalar.dma_start(gam_sb[:], gam_d[:])
            nc.scalar.dma_start(bet_sb[:], bet_d[:])


            # one register per distinct gather size
            nidx_regs = {}
            for g in range(n_groups):
                for q in range(NBUCK):
                    nbs = grp_meta[g]["sect_nb"][q]
                    if nbs and nbs * TILE not in nidx_regs:
                        nidx_regs[nbs * TILE] = nc.gpsimd.to_reg(nbs * TILE)

            for g in range(n_groups):
                gm = grp_meta[g]
                poss = gm["poss"]
                gbase = gm["blk_base"]
                gnb = gm["nblocks"]
                ncols = len(poss) * TILE

                xg = xgp.tile([TILE, max_grp_blocks * XROW], bf16, tag="xg")
                mm = mmp.tile([TILE, max_grp_blocks * TILE], mybir.dt.float8e4, tag="mm")
                idx_sb = idxp.tile([TILE, max_grp_blocks * 8], i16, tag="idx")
                h1 = h1p.tile([D + 1, GTILES * TILE], bf16, tag="h1")

                nc.sync.dma_start(idx_sb[:, :gnb * 8],
                                  idx_d[:, gbase * 8:(gbase + gnb) * 8])
                nc.sync.dma_start(mm[:, :gnb * TILE],
                                  m8_d[:, gbase * TILE:(gbase + gnb) * TILE])

                for q in range(NBUCK):
                    nbs = gm["sect_nb"][q]
                    if nbs == 0:
                        continue
                    rel = gm["sect_base"][q] - gbase
                    nrow = min(BK, N - q * BK)
                    nc.gpsimd.dma_gather(
                        out_ap=xg[:, rel * XROW:(rel + nbs) * XROW].rearrange(
                            "p (b e) -> p b e", e=XROW),
                        in_ap=x_d[q * BK:q * BK + nrow, :],
                        idxs_ap=idx_sb[:, rel * 8:(rel + nbs) * 8],
                        num_idxs=nbs * TILE,
                        num_idxs_reg=nidx_regs[nbs * TILE],
                        elem_size=XROW,
                        elem_step=XROW,
                        queue_num=q,
                        single_packet=False,
                    )

                for q in range(NBUCK):
                    nbs = gm["sect_nb"][q]
                    if nbs == 0:
                        continue
                    rel = gm["sect_base"][q] - gbase
                    sbase = gm["sect_base"][q]
                    # y = val * x into the padded half of each slot
                    nc.vector.tensor_tensor(
                        out=xg[:, rel * XROW:(rel + nbs) * XROW].rearrange(
                            "p (b e) -> p b e", e=XROW)[:, :, D:2 * D],
                        in0=xg[:, rel * XROW:(rel + nbs) * XROW].rearrange(
                            "p (b e) -> p b e", e=XROW)[:, :, 0:D],
                        in1=val_sb[:, sbase:sbase + nbs].unsqueeze(2).to_broadcast(
                            [TILE, nbs, D]),
                        op=bass.mybir.AluOpType.mult,
                    )

                for j, p in enumerate(poss):
                    ps = psA.tile([D, TILE], f32, tag="ps")
                    blks = []
                    for q in range(NBUCK):
                        nb = nb_shared[p][q]
                        bb = block_base[p][q]
                        for k in range(nb):
                            blks.append(bb - gbase + k)
                    for ki, rb in enumerate(blks):
                        nc.tensor.matmul(
                            ps[:],
                            lhsT=xg[:, rb * XROW + D:rb * XROW + 2 * D],
                            rhs=mm[:, rb * TILE:(rb + 1) * TILE],
                            start=(ki == 0),
                            stop=(ki == len(blks) - 1),
                        )
                    nc.scalar.copy(h1[0:D, j * TILE:(j + 1) * TILE], ps[:])

                # bias ones row for this group's columns (tiny DMA, off DVE)
                goff = poss[0] * TILE
                nc.scalar.dma_start(h1[D:D + 1, :ncols],
                                    ones_d[:, goff:goff + ncols])

                ps2 = psB.tile([D, GTILES * TILE], f32, tag="ps2")
                nc.tensor.matmul(
                    ps2[:, :ncols],
                    lhsT=waug_sb[:],
                    rhs=h1[:, :ncols],
                    start=True, stop=True,
                )
                nc.vector.tensor_tensor(
                    out=h3_sb[:, goff:goff + ncols],
                    in0=ps2[:, :ncols],
                    in1=xres_sb[:, goff:goff + ncols],
                    op=bass.mybir.AluOpType.add,
                )
                # BN stats for this group
                sq_scr = h1p.tile([D, GTILES * TILE], f32, tag="sq")
                nc.scalar.activation(
                    sq_scr[:, :ncols],
                    h3_sb[:, goff:goff + ncols],
                    bass.mybir.ActivationFunctionType.Square,
                    accum_out=stat_q[:, g:g + 1],
                )
                nc.vector.reduce_sum(
                    stat_s[:, g:g + 1],
                    h3_sb[:, goff:goff + ncols],
                    axis=bass.mybir.AxisListType.X,
                )

            stats2 = constp.tile([D, 2], f32)
            nc.vector.reduce_sum(stats2[:, 0:1], stat_s[:],
                                 axis=bass.mybir.AxisListType.X)
            nc.vector.reduce_sum(stats2[:, 1:2], stat_q[:],
                                 axis=bass.mybir.AxisListType.X)

            statsg = constp.tile([D, 2], f32)
            if replica_groups is not None:
                nc.gpsimd.dma_start(cc_in[:], stats2[:])
                nc.gpsimd.collective_compute(
                    "AllReduce",
                    bass.mybir.AluOpType.add,
                    replica_groups=replica_groups,
                    ins=[cc_in[:]],
                    outs=[cc_out[:]],
                )
                nc.gpsimd.dma_start(statsg[:], cc_out[:])
            else:
                nc.vector.tensor_copy(statsg[:], stats2[:])

            # finalize BN constants: A = gamma / sqrt(var + eps), Bc = beta - mean*A
            eps_sb = constp.tile([D, 1], f32)
            nc.gpsimd.memset(eps_sb[:], float(eps))
            mean = constp.tile([D, 1], f32)
            esq = constp.tile([D, 1], f32)
            var = constp.tile([D, 1], f32)
            sd = constp.tile([D, 1], f32)
            rsd = constp.tile([D, 1], f32)
            A = constp.tile([D, 1], f32)
            Bc = constp.tile([D, 1], f32)
            inv_n = 1.0 / float(N)
            nc.vector.tensor_scalar_mul(mean[:], statsg[:, 0:1], inv_n)
            nc.vector.tensor_scalar_mul(esq[:], statsg[:, 1:2], inv_n)
            nc.vector.tensor_tensor(out=var[:], in0=mean[:], in1=mean[:],
                                    op=bass.mybir.AluOpType.mult)
            nc.vector.tensor_tensor(out=var[:], in0=esq[:], in1=var[:],
                                    op=bass.mybir.AluOpType.subtract)
            nc.scalar.activation(sd[:], var[:],
                                 bass.mybir.ActivationFunctionType.Sqrt,
                                 bias=eps_sb[:, 0:1], scale=1.0)
            nc.vector.reciprocal(rsd[:], sd[:])
            nc.vector.tensor_tensor(out=A[:], in0=rsd[:], in1=gam_sb[:],
                                    op=bass.mybir.AluOpType.mult)
            nc.vector.tensor_tensor(out=Bc[:], in0=mean[:], in1=A[:],
                                    op=bass.mybir.AluOpType.mult)
            nc.vector.tensor_tensor(out=Bc[:], in0=bet_sb[:], in1=Bc[:],
                                    op=bass.mybir.AluOpType.subtract)

            # apply BN + ReLU in place, then store
            nc.scalar.activation(h3_sb[:], h3_sb[:],
                                 bass.mybir.ActivationFunctionType.Relu,
                                 bias=Bc[:, 0:1], scale=A[:, 0:1])
            nc.sync.dma_start(out_d[:], h3_sb[:])

    # Raw Bass (Tile) skips Bacc's library/ISA lowering passes; without them
    # the extended instructions (DMAGatherAnt) have empty .instr bytes and
    # walrus fails with "ISA wrong length", and no LOAD_LIB is emitted.
    import bass_rust as _bass_rust
    from concourse.library_config import all_libraries, standard
    inst_type_to_lib_mask = {}
    for lib in all_libraries:
        for inst_type in lib.instructions:
            inst_type_to_lib_mask[inst_type] = inst_type_to_lib_mask.get(
                inst_type, 0) | (1 << lib.index)
    _bass_rust.insert_library_loads(
        nc, inst_type_to_lib_mask, len(all_libraries), standard.index)
    mybir.codegen_inst_isa_subclasses(nc)
    return nc


def make_in_maps(meta, per_core, waug, x_pad, gamma, beta, n_cores):
    maps = []
    for c in range(n_cores):
        pc = per_core[c]
        maps.append({
            "x_pad": x_pad,
            "idx16": pc["idx16"],
            "val": pc["val"],
            "m8": pc["m8"],
            "xres": pc["xres"],
            "ones": pc["ones"],
            "waug": waug,
            "gam": np.asarray(gamma, dtype=np.float32).reshape(D, 1),
            "bet": np.asarray(beta, dtype=np.float32).reshape(D, 1),
        })
    return maps


# ======================================================================
# entry point
# ======================================================================
_CACHE = {}

EPS = 1e-5
N_CORES = 8


def kernel(x, adj_val, W, b, gamma, beta, adj_row, adj_col):
    install()
    x = np.asarray(x); adj_val = np.asarray(adj_val)
    W = np.asarray(W); b = np.asarray(b)
    gamma = np.asarray(gamma); beta = np.asarray(beta)
    adj_row = np.asarray(adj_row).astype(np.int64)
    adj_col = np.asarray(adj_col).astype(np.int64)

    meta, per_core, waug, x_pad = host_prep(
        x, adj_val, adj_row, adj_col, W, b, N_CORES)
    in_maps = make_in_maps(meta, per_core, waug, x_pad, gamma, beta, N_CORES)

    key = (meta["B"], tuple(tuple(v) for v in meta["nb_shared"]))
    if key not in _CACHE:
        nc = build_nc(meta, N_CORES, EPS,
                      replica_groups=[list(range(N_CORES))])
        _CACHE[key] = nc
    nc = _CACHE[key]

    from concourse.bass_utils import run_bass_kernel_spmd
    res = run_bass_kernel_spmd(nc, in_maps, list(range(N_CORES)))
    out = host_post([res.results[c]["outp"] for c in range(N_CORES)],
                    dict(meta=meta, per_core=per_core), N_CORES)
    return out.astype(np.float32)
